# revision 45
# baseline (speedup 1.0000x reference)
"""Multi-head attention (RoPE, causal) Trainium2 Bass kernel, 8 NeuronCores.

Problem: x[4,2048,1024] -> MHA(16 heads, head_dim 64, RoPE, causal mask) -> [4,2048,1024]

Sharding (pure data/tensor parallel, no collectives):
  core c -> (batch b = c//2, head-group g = c%2); each head-group = 8 heads = 512 dims.
  Each core computes q/k/v projections for its (batch, head-group), RoPE, attention,
  and a partial output projection (columns of Wo for its head group).
  Host sums the two partial outputs per batch (512-dim contraction split).

Kernel layout tricks:
  - Projections computed in transposed [out_dim, seq] layout (QT/KT) so that
    QK^T blocks come out as S^T [k, q]: softmax reductions along the partition
    dim are avoided entirely via UNSAFE softmax (no row-max; inputs are bounded
    N(0,1)-ish data, logits stay << 88).
  - Head-PAIR processing with PE array tiling: the two heads of a QTb/KTb tile
    live on partitions 0:64 / 64:128, so their K=64 QK matmuls are issued
    back-to-back as row tiles (0,0)/(64,0) and run CONCURRENTLY on the PE;
    their M=64 PV matmuls are col tiles (0,0)/(0,64) writing the two halves of
    one pv2[128, 512] accumulator, also concurrent.  ~2x attention throughput.
  - Softmax row-sums l via M=1 ones-matmuls, 4-way col-tiled (PSUM slots
    0/32/64/96 of one bank), batched over step pairs: ~1/4 pass cost.
  - Causal masking: blocks strictly below the diagonal strip computed full
    width; diagonal-strip blocks compute only columns [lo:512] (saves PE),
    with a single shared [128,128] upper-tri mask multiply on the ragged
    128-col window (split across DVE and GpSimd).  Upper blocks skipped.
  - Normalization: l partials summed + reciprocal on [1,512] rows (DVE),
    broadcast over 64 partitions via col-tiled K=1 PE matmuls, one fused
    [128,512] DVE multiply writes both heads' aT rows.
  - RoPE via a signed-permutation matrix on the TensorEngine plus 3 DVE
    elementwise ops per chunk; psum->sbuf staging copies on DVE (ACT is
    saturated by EXP).
"""

import numpy as np
import ml_dtypes

import concourse.bass as bass
import concourse.tile as tile
from concourse import bacc, mybir
from concourse import bass_utils

B, S, D, H, DH = 4, 2048, 1024, 16, 64
NCORES = 8
HG = 2              # head groups (tensor parallel)
HPG = H // HG       # heads per group = 8
OG = HPG * DH       # group output dims = 512
SCALE = DH ** -0.5
P = 128
QSB = 512           # q super-block width
NQSB = S // QSB     # 4
KB = 128            # k block
NKB = S // KB       # 16
DC = D // P         # 8 d-chunks
JC = OG // P        # 4 j-chunks (out-proj contraction)

F32 = mybir.dt.float32
F32R = mybir.dt.float32r
BF16 = mybir.dt.bfloat16

_COMPILED = {}


# ---------------------------------------------------------------- host tables

def _rope_tables():
    inv_freq = 1.0 / (10000.0 ** (np.arange(0, DH, 2, dtype=np.float32) / DH))
    t = np.arange(S, dtype=np.float32)
    freqs = np.outer(t, inv_freq).astype(np.float32)      # [S, 32]
    emb = np.concatenate([freqs, freqs], -1)              # [S, 64]
    return np.cos(emb), np.sin(emb)


def _host_consts():
    cos, sin = _rope_tables()                             # [S, 64]
    cosT2 = np.ascontiguousarray(
        np.concatenate([cos.T, cos.T], axis=0), dtype=np.float32)   # [128, S]
    sinT2 = np.ascontiguousarray(
        np.concatenate([sin.T, sin.T], axis=0), dtype=np.float32)
    # signed permutation: rot(x)[i] = -x[i+32] (j<32) else x[i-32], per 64-row head
    psig = np.zeros((P, P), np.float32)
    for i in range(P):
        j = i % DH
        base = (i // DH) * DH
        if j < 32:
            psig[i, base + j + 32] = -1.0
        else:
            psig[i, base + j - 32] = 1.0
    psigT = np.ascontiguousarray(psig.T)
    return cosT2, sinT2, psigT


def _mask_plan(mask):
    """Classify the [S, S] mask into a per-qsb block plan.

    plan[qsb] = list of (kb, msel); msel is None (no mask), ("const", r) for
    the causal diagonal-strip tiles (lo = KB*r), or ("dram", qsb, kb) for
    generic per-block mask tiles.
    """
    m = np.asarray(mask).reshape(S, S) != 0        # [q, k] True = attend
    causal = np.array_equal(m, np.tril(np.ones((S, S), bool)))
    if causal:
        plan = []
        for qsb in range(NQSB):
            row = []
            for kb in range(4 * qsb + 4):
                r = kb - 4 * qsb
                row.append((kb, None if r < 0 else ("const", r)))
            plan.append(row)
        return plan, "causal"
    if m.all():
        return [[(kb, None) for kb in range(NKB)] for _ in range(NQSB)], "full"
    plan = []
    for qsb in range(NQSB):
        row = []
        for kb in range(NKB):
            blk = m[qsb * QSB:(qsb + 1) * QSB, kb * KB:(kb + 1) * KB]  # [q, k]
            if not blk.any():
                continue          # fully masked block contributes nothing
            row.append((kb, None if blk.all() else ("dram", qsb, kb)))
        plan.append(row)
    return plan, "generic"


# ------------------------------------------------------------------- builder

def _build(plan, mode):
    nc = bacc.Bacc("TRN2", target_bir_lowering=False, debug=False, num_devices=1)
    AF = mybir.ActivationFunctionType
    OP = mybir.AluOpType

    xT_d = nc.dram_tensor("xT", [NQSB, P, DC, QSB], BF16,
                          kind="ExternalInput").ap()
    wqT_d = nc.dram_tensor("wqT", [4, P, DC, P], BF16,
                           kind="ExternalInput").ap()
    wkT_d = nc.dram_tensor("wkT", [4, P, DC, P], BF16,
                           kind="ExternalInput").ap()
    wvT_d = nc.dram_tensor("wvT", [P, DC, OG], BF16, kind="ExternalInput").ap()
    woT_d = nc.dram_tensor("woT", [8, P, JC, P], BF16,
                           kind="ExternalInput").ap()
    cos_d = nc.dram_tensor("cosT", [P, S], BF16, kind="ExternalInput").ap()
    sin_d = nc.dram_tensor("sinT", [P, S], BF16, kind="ExternalInput").ap()
    psg_d = nc.dram_tensor("psgT", [P, P], BF16, kind="ExternalInput").ap()
    if mode == "causal":
        mk_d = nc.dram_tensor("mask128", [P, KB], BF16, kind="ExternalInput").ap()
    elif mode == "generic":
        m01_d = nc.dram_tensor("m01", [NQSB, NKB, P, QSB], F32,
                               kind="ExternalInput").ap()
    outT_d = nc.dram_tensor("outT", [D, S], BF16, kind="ExternalOutput").ap()
    outB_d = nc.dram_tensor("outB", [D, S], BF16, kind="ExternalOutput").ap()

    with tile.TileContext(nc) as tc:
        from contextlib import ExitStack
        with ExitStack() as ctx:
            persist = ctx.enter_context(tc.tile_pool(name="persist", bufs=1))
            wstream = ctx.enter_context(tc.tile_pool(name="wstream", bufs=2))
            work = ctx.enter_context(tc.tile_pool(name="work", bufs=2))
            prepool = ctx.enter_context(tc.tile_pool(name="prepool", bufs=2))
            ptpool = ctx.enter_context(tc.tile_pool(name="ptpool", bufs=6))
            nrmpool = ctx.enter_context(tc.tile_pool(name="nrmpool", bufs=2))
            stp = ctx.enter_context(
                tc.tile_pool(name="stp", bufs=2, space="PSUM"))
            spp = ctx.enter_context(
                tc.tile_pool(name="spp", bufs=2, space="PSUM"))
            pvp = ctx.enter_context(
                tc.tile_pool(name="pvp", bufs=2, space="PSUM"))

            # bf16 post-rope Q/K and bf16 V (with ones column) live all-kernel
            QTb = [persist.tile([P, S], BF16, tag=f"qt{t}", name=f"qtb{t}")
                   for t in range(4)]
            KTb = [persist.tile([P, S], BF16, tag=f"kt{t}", name=f"ktb{t}")
                   for t in range(4)]
            V = [persist.tile([P, HPG, DH + 1], BF16, tag=f"v{sb}",
                              name=f"v{sb}") for sb in range(NKB)]
            for sb in range(NKB):
                nc.vector.memset(V[sb][:, :, DH:DH + 1], 1.0)

            # prefetch tile-0 Q/K weight chunks and the first x chunk first
            # on their queues so the first projection matmuls start after a
            # fraction of the ~15 MB bulk DMA
            xTs = [persist.tile([P, DC, QSB], BF16, tag=f"xt{sc}",
                                 name=f"xt{sc}") for sc in range(4)]
            nc.scalar.dma_start(xTs[0][:, 0:2, :], xT_d[0][:, 0:2, :])
            nc.gpsimd.dma_start(xTs[0][:, 2:4, :], xT_d[0][:, 2:4, :])
            nc.scalar.dma_start(xTs[0][:, 4:6, :], xT_d[0][:, 4:6, :])
            nc.gpsimd.dma_start(xTs[0][:, 6:DC, :], xT_d[0][:, 6:DC, :])
            wqk_live = {}
            for who, w_d in (("q", wqT_d), ("k", wkT_d)):
                w_oc = wstream.tile([P, DC, P], BF16, tag="wqk",
                                    name=f"w{who}0")
                nc.sync.dma_start(w_oc[:, 0:4, :], w_d[0][:, 0:4, :])
                nc.sync.dma_start(w_oc[:, 4:DC, :], w_d[0][:, 4:DC, :])
                wqk_live[who] = w_oc
            psg_sb = persist.tile([P, P], BF16, tag="psg")
            nc.sync.dma_start(psg_sb[:], psg_d)
            wv = persist.tile([P, DC, OG], BF16, tag="wv")
            nc.gpsimd.dma_start(wv[:], wvT_d)
            cos_sb = persist.tile([P, S], BF16, tag="cos")
            sin_sb = persist.tile([P, S], BF16, tag="sin")
            nc.gpsimd.dma_start(cos_sb[:], cos_d)
            nc.gpsimd.dma_start(sin_sb[:], sin_d)
            for sc in range(1, 4):
                nc.sync.dma_start(xTs[sc][:], xT_d[sc])
            aT = [persist.tile([P, S], BF16, tag=f"at{t}", name=f"at{t}")
                  for t in range(4)]
            ones64 = persist.tile([1, DH], BF16, tag="ones64")
            nc.vector.memset(ones64[:], 1.0)
            if mode == "causal":
                mk128 = persist.tile([P, KB], BF16, tag="mk128")
                nc.gpsimd.dma_start(mk128[:], mk_d)

            # ---------------- emitters (generators) ----------------
            # yield points let attention steps and projection halves weave at
            # ~1 us granularity so the PE never sees an ACT-bound stretch

            def gen_qk_unit(w_d, dst, oc, sc, who):
                """One [128, 512] chunk of a Q/K projection + RoPE (2 steps)."""
                if sc == 0 and oc > 0:
                    w_oc = wstream.tile([P, DC, P], BF16, tag="wqk",
                                        name=f"w{who}{oc}")
                    nc.sync.dma_start(w_oc[:], w_d[oc])
                    wqk_live[who] = w_oc
                w_oc = wqk_live[who]
                sl = slice(sc * QSB, (sc + 1) * QSB)
                ps = spp.tile([P, QSB], F32, tag="sp", name="ps")
                for dc in range(4):
                    nc.tensor.matmul(
                        ps[:], w_oc[:, dc, :], xTs[sc][:, dc, :],
                        start=(dc == 0), stop=False)
                yield
                for dc in range(4, DC):
                    nc.tensor.matmul(
                        ps[:], w_oc[:, dc, :], xTs[sc][:, dc, :],
                        start=False, stop=(dc == DC - 1))
                pre = prepool.tile([P, QSB], BF16, tag="pre")
                nc.vector.tensor_copy(pre[:], ps[:])
                rot = spp.tile([P, QSB], F32, tag="sp", name="rot")
                nc.tensor.matmul(rot[:], psg_sb[:], pre[:],
                                 start=True, stop=True)
                m = work.tile([P, QSB], BF16, tag="ropem")
                nc.gpsimd.tensor_tensor(m[:], pre[:], cos_sb[:, sl], OP.mult)
                nc.vector.tensor_tensor(
                    dst[oc][:, sl], rot[:], sin_sb[:, sl], OP.mult)
                nc.gpsimd.tensor_tensor(
                    dst[oc][:, sl], dst[oc][:, sl], m[:], OP.add)
                yield

            def gen_v_unit(sb):
                ps = spp.tile([P, QSB], F32, tag="sp", name="ps")
                xsc, xo = sb // 4, (sb % 4) * P
                for dc in range(4):
                    nc.tensor.matmul(
                        ps[:], xTs[xsc][:, dc, xo:xo + P], wv[:, dc, :],
                        start=(dc == 0), stop=False)
                yield
                for dc in range(4, DC):
                    nc.tensor.matmul(
                        ps[:], xTs[xsc][:, dc, xo:xo + P], wv[:, dc, :],
                        start=False, stop=(dc == DC - 1))
                nc.scalar.copy(
                    V[sb][:, :, 0:DH],
                    ps[:].rearrange("p (h j) -> p h j", j=DH))
                yield

            wos = []

            def gen_op_unit(oc, sc, jlo, jhi, dest):
                """Half of an out-proj psum group (contraction jc in [jlo,jhi))."""
                ssl = slice(sc * QSB, (sc + 1) * QSB)
                ps = spp.tile([P, QSB], F32, tag="sp", name="ps")
                for jc in range(jlo, jhi):
                    nc.tensor.matmul(
                        ps[:], wos[oc][:, jc, :], aT[jc][:, ssl],
                        start=(jc == jlo), stop=(jc == jhi - 1))
                stg = work.tile([P, QSB], BF16, tag="stg", bufs=3, name="stg")
                nc.vector.tensor_copy(stg[:], ps[:])
                e1, e2 = ((nc.sync, nc.scalar), (nc.scalar, nc.sync))[oc % 2]
                half = QSB // 2
                e1.dma_start(dest[oc * P:(oc + 1) * P,
                                  sc * QSB:sc * QSB + half],
                             stg[:, 0:half])
                e2.dma_start(dest[oc * P:(oc + 1) * P,
                                  sc * QSB + half:(sc + 1) * QSB],
                             stg[:, half:QSB])
                yield

            pending_norm = []

            def flush_norm():
                while pending_norm:
                    pending_norm.pop(0)()

            def gen_attn_group(t, qsb):
                """Attention for head pair (2t, 2t+1) on q super-block qsb.

                Software-pipelined: step si issues the row-tiled QK pair and
                exps for block si, then the col-tiled PV pair + 2-way l
                ones-matmuls for block si-1 (whose inputs are all long ready,
                so the scheduler keeps each pair adjacent -> PE concurrency).
                """
                h0, h1 = 2 * t, 2 * t + 1
                qoff = qsb * QSB
                qsl = slice(qoff, qoff + QSB)
                blocks = plan[qsb]
                L = len(blocks)
                state = {}
                recs = []           # si -> (pt0, pt1, lo, kb)

                def emit_pv(s):
                    pt2, lo, kb = recs[s]
                    if s == 0:
                        # run the previous group's deferred normalization
                        # before its pv psum slots are recycled
                        flush_norm()
                        state["pva"] = pvp.tile([DH + 1, QSB], F32,
                                                tag="pv", name="pva")
                        state["pvb"] = pvp.tile([DH + 1, QSB], F32,
                                                tag="pv", name="pvb")
                    first, last = s == 0, s == L - 1
                    nc.tensor.matmul(
                        state["pva"][:, lo:QSB], V[kb][:, h0, :],
                        pt2[:, 0, lo:QSB], start=first, stop=last)
                    nc.tensor.matmul(
                        state["pvb"][:, lo:QSB], V[kb][:, h1, :],
                        pt2[:, 1, lo:QSB], start=first, stop=last)

                for si, (kb, msel) in enumerate(blocks):
                    lo = 0
                    generic_m = None
                    if msel is not None:
                        if msel[0] == "const":
                            lo = KB * msel[1]
                        else:
                            generic_m = msel
                    ksl = slice(kb * KB, (kb + 1) * KB)
                    qlo = slice(qoff + lo, qoff + QSB)
                    st2 = stp.tile([P, 2, QSB], F32, tag="st", name="st2")
                    nc.tensor.matmul(
                        st2[:, 0, lo:QSB], KTb[t][0:DH, ksl],
                        QTb[t][0:DH, qlo],
                        start=True, stop=True, tile_position=(0, 0))
                    nc.tensor.matmul(
                        st2[:, 1, lo:QSB], KTb[t][DH:P, ksl],
                        QTb[t][DH:P, qlo],
                        start=True, stop=True, tile_position=(DH, 0))
                    pt2 = ptpool.tile([P, 2, QSB], BF16, tag="pt", name="pt2")
                    nc.scalar.activation(
                        pt2[:, 0:2, lo:QSB], st2[:, 0:2, lo:QSB], AF.Exp,
                        scale=SCALE)
                    if msel is not None and msel[0] == "const":
                        # ragged 128-col window gets the shared tri-mask
                        # (GpSimd: pt/mk are SBUF, keeps DVE free)
                        w = slice(lo, lo + KB)
                        nc.gpsimd.tensor_tensor(pt2[:, 0, w], pt2[:, 0, w],
                                                mk128[:], OP.mult)
                        nc.gpsimd.tensor_tensor(pt2[:, 1, w], pt2[:, 1, w],
                                                mk128[:], OP.mult)
                    elif generic_m is not None:
                        mg = work.tile([P, QSB], F32, tag="mg")
                        nc.sync.dma_start(mg[:], m01_d[generic_m[1],
                                                       generic_m[2]])
                        mgb = work.tile([P, QSB], BF16, tag="mgb")
                        nc.vector.tensor_copy(mgb[:], mg[:])
                        nc.vector.tensor_tensor(pt2[:, 0, :], pt2[:, 0, :],
                                                mgb[:], OP.mult)
                        nc.vector.tensor_tensor(pt2[:, 1, :], pt2[:, 1, :],
                                                mgb[:], OP.mult)
                    recs.append((pt2, lo, kb))
                    if si >= 1:
                        emit_pv(si - 1)
                    yield
                emit_pv(L - 1)

                # row-sums l live in pv row 64 (V ones column); bf16 copies
                # feed the PE broadcast in the deferred norm
                lrb0 = nrmpool.tile([1, QSB], BF16, tag="lr0", name="lr0")
                lrb1 = nrmpool.tile([1, QSB], BF16, tag="lr1", name="lr1")
                pva, pvb = state["pva"], state["pvb"]
                nc.vector.tensor_copy(lrb0[:], pva[DH:DH + 1, :])
                nc.vector.tensor_copy(lrb1[:], pvb[DH:DH + 1, :])

                def _norm(t=t, qsl=qsl, pva=pva, pvb=pvb, lrb0=lrb0,
                          lrb1=lrb1):
                    # col-tiled pair broadcast of l, then per-head staging
                    # (ACT for h0; DVE partition-shift copy for h1), 1/l, mult
                    bc = spp.tile([P, QSB], F32, tag="sp", name="bc")
                    nc.tensor.matmul(bc[0:DH, :], ones64[:], lrb0[:],
                                     start=True, stop=True,
                                     tile_position=(0, 0))
                    nc.tensor.matmul(bc[DH:P, :], ones64[:], lrb1[:],
                                     start=True, stop=True,
                                     tile_position=(0, DH))
                    bcS0 = work.tile([DH, QSB], F32, tag="bcS",
                                     name="bcS0", bufs=3)
                    bcS1 = work.tile([DH, QSB], F32, tag="bcT",
                                     name="bcS1", bufs=3)
                    nc.scalar.copy(bcS0[:], bc[0:DH, :])
                    nc.vector.tensor_copy(bcS1[:], bc[DH:P, :])
                    for ph, pv, bcS in ((0, pva, bcS0), (DH, pvb, bcS1)):
                        rS = work.tile([DH, QSB], F32, tag="rS",
                                       name="rS", bufs=3)
                        nc.vector.reciprocal_approx_fast(rS[:], bcS[:])
                        nc.vector.tensor_tensor(
                            aT[t][ph:ph + DH, qsl], pv[0:DH, :], rS[:],
                            OP.mult)
                pending_norm.append(_norm)
                yield

            def drain(g):
                for _ in g:
                    pass

            def chain(gens):
                for g in gens:
                    yield from g

            def weave(agen, pgen, ratio):
                """Drain agen; after each yield, advance pgen by `ratio`."""
                acc = 0.0
                alive = True
                for _ in agen:
                    if not alive:
                        continue
                    acc += ratio
                    while acc >= 1.0:
                        if next(pgen, _SENT) is _SENT:
                            alive = False
                            break
                        acc -= 1.0
                for _ in pgen:
                    pass

            _SENT = object()

            # ---------------- interleaved emission ----------------
            # tile 0 projections + all of V up front (V feeds every round,
            # and trace order defines the dependency semantics); grouped by
            # x chunk so early units only wait on early DMA arrivals
            for sc in range(4):
                drain(gen_qk_unit(wqT_d, QTb, 0, sc, "q"))
                drain(gen_qk_unit(wkT_d, KTb, 0, sc, "k"))
                for sb in range(4 * sc, 4 * sc + 4):
                    drain(gen_v_unit(sb))

            # rounds: attention for head pair t woven with tile t+1
            # projections / (round 3) the first half of the output projection,
            # so the PE always has ACT-free matmul work within a HAM window
            n_ayield = sum(len(row) + 1 for row in plan)
            for t in range(4):
                if t == 1:
                    # prefetch all out-proj weights (needed from round 2 on)
                    for oc in range(8):
                        wo = wstream.tile([P, JC, P], BF16, tag="wo", bufs=8,
                                          name=f"wo{oc}")
                        nc.sync.dma_start(wo[:], woT_d[oc])
                        wos.append(wo)
                if t < 3:
                    agen = chain([gen_attn_group(t, qsb)
                                  for qsb in range(NQSB)])
                    pgens = []
                    for sc in range(4):
                        pgens.append(gen_qk_unit(wqT_d, QTb, t + 1, sc, "q"))
                    for sc in range(4):
                        pgens.append(gen_qk_unit(wkT_d, KTb, t + 1, sc, "k"))
                    n_p = 16
                    if t == 2:
                        # outT half (jc 0..2): aT[0]/aT[1] are final now
                        pgens += [gen_op_unit(oc, sc, 0, 2, outT_d)
                                  for oc in range(8) for sc in range(4)]
                        n_p += 32
                    weave(agen, chain(pgens), float(n_p) / n_ayield)
                else:
                    # round 3: outB units (jc 2..4) for q-chunk sc follow
                    # group sc+1 (whose start flushed sc's aT norm)
                    def agen3():
                        for qsb in range(NQSB):
                            yield from gen_attn_group(3, qsb)
                            if qsb >= 1:
                                for oc in range(8):
                                    yield from gen_op_unit(
                                        oc, qsb - 1, 2, JC, outB_d)
                    drain(agen3())

            flush_norm()
            # last outB q-chunk (aT[3] tail)
            for oc in range(8):
                drain(gen_op_unit(oc, 3, 2, JC, outB_d))

    nc.compile()
    return nc


def _plan_key(plan, mode):
    return (mode, tuple(tuple(row) for row in plan))


def _get_compiled(mask):
    plan, mode = _mask_plan(mask)
    key = _plan_key(plan, mode)
    if key not in _COMPILED:
        _COMPILED[key] = (_build(plan, mode), plan, mode)
    return _COMPILED[key]


# --------------------------------------------------------------- host driver

def _make_in_maps(x, Wq, Wk, Wv, Wo, mask, mode):
    cosT2, sinT2, psigT = _host_consts()
    consts = {"cosT": cosT2.astype(ml_dtypes.bfloat16),
              "sinT": sinT2.astype(ml_dtypes.bfloat16),
              "psgT": psigT.astype(ml_dtypes.bfloat16)}
    if mode == "causal":
        # mask128[k, q'] = 1 where q' >= k (ragged diag window, shared)
        consts["mask128"] = np.triu(
            np.ones((P, KB), np.float32)).astype(ml_dtypes.bfloat16)
    elif mode == "generic":
        m = (np.asarray(mask).reshape(S, S) != 0)
        m01 = np.zeros((NQSB, NKB, P, QSB), np.float32)
        for qsb in range(NQSB):
            for kb in range(NKB):
                blk = m[qsb * QSB:(qsb + 1) * QSB, kb * KB:(kb + 1) * KB]
                m01[qsb, kb] = blk.T.astype(np.float32)
        consts["m01"] = m01

    def arr_qk(w):
        # [D, OG_rows] -> per-oc [P, DC, P]: wT[d, o] laid out [oc, p(o), dc, o']
        wT = w.T.astype(np.float32)                       # [D, OG]
        a = wT.reshape(DC, P, 4, P)          # [dc, p(d), oc, o']
        return np.ascontiguousarray(a.transpose(2, 1, 0, 3)).astype(
            ml_dtypes.bfloat16)

    in_maps = []
    for c in range(NCORES):
        b, g = c // HG, c % HG
        rows = slice(OG * g, OG * (g + 1))
        xT = x[b].T.astype(np.float32)                    # [D, S]
        xTa = np.ascontiguousarray(
            xT.reshape(DC, P, NQSB, QSB).transpose(2, 1, 0, 3)).astype(
                ml_dtypes.bfloat16)
        wq = arr_qk(Wq[rows, :])
        wk = arr_qk(Wk[rows, :])
        wvT = np.ascontiguousarray(
            Wv[rows, :].T.astype(np.float32).reshape(DC, P, OG)
            .transpose(1, 0, 2)).astype(ml_dtypes.bfloat16)
        woT = Wo[:, rows].T.astype(np.float32)            # [OG, D]
        woa = np.ascontiguousarray(
            woT.reshape(JC, P, 8, P).transpose(2, 1, 0, 3)
        ).astype(ml_dtypes.bfloat16)
        in_maps.append({
            "xT": xTa,
            "wqT": wq,
            "wkT": wk,
            "wvT": wvT,
            "woT": woa,
            **consts,
        })
    return in_maps


def run(x, Wq, Wk, Wv, Wo, mask, trace=False):
    nc, plan, mode = _get_compiled(mask)
    in_maps = _make_in_maps(x, Wq, Wk, Wv, Wo, mask, mode)
    res = bass_utils.run_bass_kernel_spmd(
        nc, in_maps, core_ids=list(range(NCORES)), trace=trace)
    out = np.empty((B, S, D), np.float32)
    for b in range(B):
        acc = (res.results[2 * b]["outT"].astype(np.float32)
               + res.results[2 * b]["outB"].astype(np.float32)
               + res.results[2 * b + 1]["outT"].astype(np.float32)
               + res.results[2 * b + 1]["outB"].astype(np.float32))
        out[b] = acc.T
    return out, res


def kernel(x, Wq, Wk, Wv, Wo, mask):
    x = np.asarray(x, dtype=np.float32)
    Wq = np.asarray(Wq, dtype=np.float32)
    Wk = np.asarray(Wk, dtype=np.float32)
    Wv = np.asarray(Wv, dtype=np.float32)
    Wo = np.asarray(Wo, dtype=np.float32)
    out, _ = run(x, Wq, Wk, Wv, Wo, mask)
    return out


# revision 46
# speedup vs baseline: 1.0078x; 1.0078x over previous
"""Multi-head attention (RoPE, causal) Trainium2 Bass kernel, 8 NeuronCores.

Problem: x[4,2048,1024] -> MHA(16 heads, head_dim 64, RoPE, causal mask) -> [4,2048,1024]

Sharding (pure data/tensor parallel, no collectives):
  core c -> (batch b = c//2, head-group g = c%2); each head-group = 8 heads = 512 dims.
  Each core computes q/k/v projections for its (batch, head-group), RoPE, attention,
  and a partial output projection (columns of Wo for its head group).
  Host sums the two partial outputs per batch (512-dim contraction split).

Kernel layout tricks:
  - Projections computed in transposed [out_dim, seq] layout (QT/KT) so that
    QK^T blocks come out as S^T [k, q]: softmax reductions along the partition
    dim are avoided entirely via UNSAFE softmax (no row-max; inputs are bounded
    N(0,1)-ish data, logits stay << 88).
  - Head-PAIR processing with PE array tiling: the two heads of a QTb/KTb tile
    live on partitions 0:64 / 64:128, so their K=64 QK matmuls are issued
    back-to-back as row tiles (0,0)/(64,0) and run CONCURRENTLY on the PE;
    their M=64 PV matmuls are col tiles (0,0)/(0,64) writing the two halves of
    one pv2[128, 512] accumulator, also concurrent.  ~2x attention throughput.
  - Softmax row-sums l via M=1 ones-matmuls, 4-way col-tiled (PSUM slots
    0/32/64/96 of one bank), batched over step pairs: ~1/4 pass cost.
  - Causal masking: blocks strictly below the diagonal strip computed full
    width; diagonal-strip blocks compute only columns [lo:512] (saves PE),
    with a single shared [128,128] upper-tri mask multiply on the ragged
    128-col window (split across DVE and GpSimd).  Upper blocks skipped.
  - Normalization: l partials summed + reciprocal on [1,512] rows (DVE),
    broadcast over 64 partitions via col-tiled K=1 PE matmuls, one fused
    [128,512] DVE multiply writes both heads' aT rows.
  - RoPE via a signed-permutation matrix on the TensorEngine plus 3 DVE
    elementwise ops per chunk; psum->sbuf staging copies on DVE (ACT is
    saturated by EXP).
"""

import numpy as np
import ml_dtypes

import concourse.bass as bass
import concourse.tile as tile
from concourse import bacc, mybir
from concourse import bass_utils

B, S, D, H, DH = 4, 2048, 1024, 16, 64
NCORES = 8
HG = 2              # head groups (tensor parallel)
HPG = H // HG       # heads per group = 8
OG = HPG * DH       # group output dims = 512
SCALE = DH ** -0.5
P = 128
QSB = 512           # q super-block width
NQSB = S // QSB     # 4
KB = 128            # k block
NKB = S // KB       # 16
DC = D // P         # 8 d-chunks
JC = OG // P        # 4 j-chunks (out-proj contraction)

F32 = mybir.dt.float32
F32R = mybir.dt.float32r
BF16 = mybir.dt.bfloat16

_COMPILED = {}


# ---------------------------------------------------------------- host tables

def _rope_tables():
    inv_freq = 1.0 / (10000.0 ** (np.arange(0, DH, 2, dtype=np.float32) / DH))
    t = np.arange(S, dtype=np.float32)
    freqs = np.outer(t, inv_freq).astype(np.float32)      # [S, 32]
    emb = np.concatenate([freqs, freqs], -1)              # [S, 64]
    return np.cos(emb), np.sin(emb)


def _host_consts():
    cos, sin = _rope_tables()                             # [S, 64]
    cosT2 = np.ascontiguousarray(
        np.concatenate([cos.T, cos.T], axis=0), dtype=np.float32)   # [128, S]
    sinT2 = np.ascontiguousarray(
        np.concatenate([sin.T, sin.T], axis=0), dtype=np.float32)
    # signed permutation: rot(x)[i] = -x[i+32] (j<32) else x[i-32], per 64-row head
    psig = np.zeros((P, P), np.float32)
    for i in range(P):
        j = i % DH
        base = (i // DH) * DH
        if j < 32:
            psig[i, base + j + 32] = -1.0
        else:
            psig[i, base + j - 32] = 1.0
    psigT = np.ascontiguousarray(psig.T)
    return cosT2, sinT2, psigT


def _mask_plan(mask):
    """Classify the [S, S] mask into a per-qsb block plan.

    plan[qsb] = list of (kb, msel); msel is None (no mask), ("const", r) for
    the causal diagonal-strip tiles (lo = KB*r), or ("dram", qsb, kb) for
    generic per-block mask tiles.
    """
    m = np.asarray(mask).reshape(S, S) != 0        # [q, k] True = attend
    causal = np.array_equal(m, np.tril(np.ones((S, S), bool)))
    if causal:
        plan = []
        for qsb in range(NQSB):
            row = []
            for kb in range(4 * qsb + 4):
                r = kb - 4 * qsb
                row.append((kb, None if r < 0 else ("const", r)))
            plan.append(row)
        return plan, "causal"
    if m.all():
        return [[(kb, None) for kb in range(NKB)] for _ in range(NQSB)], "full"
    plan = []
    for qsb in range(NQSB):
        row = []
        for kb in range(NKB):
            blk = m[qsb * QSB:(qsb + 1) * QSB, kb * KB:(kb + 1) * KB]  # [q, k]
            if not blk.any():
                continue          # fully masked block contributes nothing
            row.append((kb, None if blk.all() else ("dram", qsb, kb)))
        plan.append(row)
    return plan, "generic"


# ------------------------------------------------------------------- builder

def _build(plan, mode):
    nc = bacc.Bacc("TRN2", target_bir_lowering=False, debug=False, num_devices=1)
    AF = mybir.ActivationFunctionType
    OP = mybir.AluOpType

    xT_d = nc.dram_tensor("xT", [NQSB, P, DC, QSB], BF16,
                          kind="ExternalInput").ap()
    wqT_d = nc.dram_tensor("wqT", [4, P, DC, P], BF16,
                           kind="ExternalInput").ap()
    wkT_d = nc.dram_tensor("wkT", [4, P, DC, P], BF16,
                           kind="ExternalInput").ap()
    wvT_d = nc.dram_tensor("wvT", [P, DC, OG], BF16, kind="ExternalInput").ap()
    woT_d = nc.dram_tensor("woT", [8, P, JC, P], BF16,
                           kind="ExternalInput").ap()
    cos_d = nc.dram_tensor("cosT", [P, S], BF16, kind="ExternalInput").ap()
    sin_d = nc.dram_tensor("sinT", [P, S], BF16, kind="ExternalInput").ap()
    psg_d = nc.dram_tensor("psgT", [P, P], BF16, kind="ExternalInput").ap()
    if mode == "causal":
        mk_d = nc.dram_tensor("mask128", [P, KB], BF16, kind="ExternalInput").ap()
    elif mode == "generic":
        m01_d = nc.dram_tensor("m01", [NQSB, NKB, P, QSB], F32,
                               kind="ExternalInput").ap()
    outT_d = nc.dram_tensor("outT", [D, S], BF16, kind="ExternalOutput").ap()
    outB_d = nc.dram_tensor("outB", [D, S], BF16, kind="ExternalOutput").ap()

    with tile.TileContext(nc) as tc:
        from contextlib import ExitStack
        with ExitStack() as ctx:
            persist = ctx.enter_context(tc.tile_pool(name="persist", bufs=1))
            wstream = ctx.enter_context(tc.tile_pool(name="wstream", bufs=2))
            work = ctx.enter_context(tc.tile_pool(name="work", bufs=2))
            prepool = ctx.enter_context(tc.tile_pool(name="prepool", bufs=2))
            ptpool = ctx.enter_context(tc.tile_pool(name="ptpool", bufs=6))
            nrmpool = ctx.enter_context(tc.tile_pool(name="nrmpool", bufs=2))
            stp = ctx.enter_context(
                tc.tile_pool(name="stp", bufs=2, space="PSUM"))
            spp = ctx.enter_context(
                tc.tile_pool(name="spp", bufs=2, space="PSUM"))
            pvp = ctx.enter_context(
                tc.tile_pool(name="pvp", bufs=2, space="PSUM"))

            # bf16 post-rope Q/K and bf16 V (with ones column) live all-kernel
            QTb = [persist.tile([P, S], BF16, tag=f"qt{t}", name=f"qtb{t}")
                   for t in range(4)]
            KTb = [persist.tile([P, S], BF16, tag=f"kt{t}", name=f"ktb{t}")
                   for t in range(4)]
            V = [persist.tile([P, HPG, DH + 1], BF16, tag=f"v{sb}",
                              name=f"v{sb}") for sb in range(NKB)]
            for sb in range(NKB):
                nc.vector.memset(V[sb][:, :, DH:DH + 1], 1.0)

            # prefetch tile-0 Q/K weight chunks and the first x chunk first
            # on their queues so the first projection matmuls start after a
            # fraction of the ~15 MB bulk DMA
            xTs = [persist.tile([P, DC, QSB], BF16, tag=f"xt{sc}",
                                 name=f"xt{sc}") for sc in range(4)]
            nc.scalar.dma_start(xTs[0][:, 0:4, :], xT_d[0][:, 0:4, :])
            nc.gpsimd.dma_start(xTs[0][:, 4:DC, :], xT_d[0][:, 4:DC, :])
            wqk_live = {}
            for who, w_d in (("q", wqT_d), ("k", wkT_d)):
                w_oc = wstream.tile([P, DC, P], BF16, tag="wqk",
                                    name=f"w{who}0")
                nc.sync.dma_start(w_oc[:, 0:4, :], w_d[0][:, 0:4, :])
                nc.sync.dma_start(w_oc[:, 4:DC, :], w_d[0][:, 4:DC, :])
                wqk_live[who] = w_oc
            psg_sb = persist.tile([P, P], BF16, tag="psg")
            nc.sync.dma_start(psg_sb[:], psg_d)
            wv = persist.tile([P, DC, OG], BF16, tag="wv")
            nc.gpsimd.dma_start(wv[:], wvT_d)
            cos_sb = persist.tile([P, S], BF16, tag="cos")
            sin_sb = persist.tile([P, S], BF16, tag="sin")
            nc.gpsimd.dma_start(cos_sb[:], cos_d)
            nc.gpsimd.dma_start(sin_sb[:], sin_d)
            for sc in range(1, 4):
                nc.sync.dma_start(xTs[sc][:], xT_d[sc])
            aT = [persist.tile([P, S], BF16, tag=f"at{t}", name=f"at{t}")
                  for t in range(4)]
            ones64 = persist.tile([1, DH], BF16, tag="ones64")
            nc.vector.memset(ones64[:], 1.0)
            if mode == "causal":
                mk128 = persist.tile([P, KB], BF16, tag="mk128")
                nc.gpsimd.dma_start(mk128[:], mk_d)

            # ---------------- emitters (generators) ----------------
            # yield points let attention steps and projection halves weave at
            # ~1 us granularity so the PE never sees an ACT-bound stretch

            def gen_qk_unit(w_d, dst, oc, sc, who):
                """One [128, 512] chunk of a Q/K projection + RoPE (2 steps)."""
                if sc == 0 and oc > 0:
                    w_oc = wstream.tile([P, DC, P], BF16, tag="wqk",
                                        name=f"w{who}{oc}")
                    nc.sync.dma_start(w_oc[:], w_d[oc])
                    wqk_live[who] = w_oc
                w_oc = wqk_live[who]
                sl = slice(sc * QSB, (sc + 1) * QSB)
                ps = spp.tile([P, QSB], F32, tag="sp", name="ps")
                for dc in range(4):
                    nc.tensor.matmul(
                        ps[:], w_oc[:, dc, :], xTs[sc][:, dc, :],
                        start=(dc == 0), stop=False)
                yield
                for dc in range(4, DC):
                    nc.tensor.matmul(
                        ps[:], w_oc[:, dc, :], xTs[sc][:, dc, :],
                        start=False, stop=(dc == DC - 1))
                pre = prepool.tile([P, QSB], BF16, tag="pre")
                nc.vector.tensor_copy(pre[:], ps[:])
                rot = spp.tile([P, QSB], F32, tag="sp", name="rot")
                nc.tensor.matmul(rot[:], psg_sb[:], pre[:],
                                 start=True, stop=True)
                m = work.tile([P, QSB], BF16, tag="ropem")
                nc.gpsimd.tensor_tensor(m[:], pre[:], cos_sb[:, sl], OP.mult)
                nc.vector.tensor_tensor(
                    dst[oc][:, sl], rot[:], sin_sb[:, sl], OP.mult)
                nc.gpsimd.tensor_tensor(
                    dst[oc][:, sl], dst[oc][:, sl], m[:], OP.add)
                yield

            def gen_v_unit(sb):
                ps = spp.tile([P, QSB], F32, tag="sp", name="ps")
                xsc, xo = sb // 4, (sb % 4) * P
                for dc in range(4):
                    nc.tensor.matmul(
                        ps[:], xTs[xsc][:, dc, xo:xo + P], wv[:, dc, :],
                        start=(dc == 0), stop=False)
                yield
                for dc in range(4, DC):
                    nc.tensor.matmul(
                        ps[:], xTs[xsc][:, dc, xo:xo + P], wv[:, dc, :],
                        start=False, stop=(dc == DC - 1))
                nc.vector.tensor_copy(
                    V[sb][:, :, 0:DH],
                    ps[:].rearrange("p (h j) -> p h j", j=DH))
                yield

            wos = []

            def gen_op_unit(oc, sc, jlo, jhi, dest):
                """Half of an out-proj psum group (contraction jc in [jlo,jhi))."""
                ssl = slice(sc * QSB, (sc + 1) * QSB)
                ps = spp.tile([P, QSB], F32, tag="sp", name="ps")
                for jc in range(jlo, jhi):
                    nc.tensor.matmul(
                        ps[:], wos[oc][:, jc, :], aT[jc][:, ssl],
                        start=(jc == jlo), stop=(jc == jhi - 1))
                stg = work.tile([P, QSB], BF16, tag="stg", bufs=3, name="stg")
                nc.vector.tensor_copy(stg[:], ps[:])
                e1, e2 = ((nc.sync, nc.scalar), (nc.scalar, nc.sync))[oc % 2]
                half = QSB // 2
                e1.dma_start(dest[oc * P:(oc + 1) * P,
                                  sc * QSB:sc * QSB + half],
                             stg[:, 0:half])
                e2.dma_start(dest[oc * P:(oc + 1) * P,
                                  sc * QSB + half:(sc + 1) * QSB],
                             stg[:, half:QSB])
                yield

            pending_norm = []

            def flush_norm():
                while pending_norm:
                    pending_norm.pop(0)()

            def gen_attn_group(t, qsb):
                """Attention for head pair (2t, 2t+1) on q super-block qsb.

                Software-pipelined: step si issues the row-tiled QK pair and
                exps for block si, then the col-tiled PV pair + 2-way l
                ones-matmuls for block si-1 (whose inputs are all long ready,
                so the scheduler keeps each pair adjacent -> PE concurrency).
                """
                h0, h1 = 2 * t, 2 * t + 1
                qoff = qsb * QSB
                qsl = slice(qoff, qoff + QSB)
                blocks = plan[qsb]
                L = len(blocks)
                state = {}
                recs = []           # si -> (pt0, pt1, lo, kb)

                def emit_pv(s):
                    pt2, lo, kb = recs[s]
                    if s == 0:
                        # run the previous group's deferred normalization
                        # before its pv psum slots are recycled
                        flush_norm()
                        state["pva"] = pvp.tile([DH + 1, QSB], F32,
                                                tag="pv", name="pva")
                        state["pvb"] = pvp.tile([DH + 1, QSB], F32,
                                                tag="pv", name="pvb")
                    first, last = s == 0, s == L - 1
                    nc.tensor.matmul(
                        state["pva"][:, lo:QSB], V[kb][:, h0, :],
                        pt2[:, 0, lo:QSB], start=first, stop=last)
                    nc.tensor.matmul(
                        state["pvb"][:, lo:QSB], V[kb][:, h1, :],
                        pt2[:, 1, lo:QSB], start=first, stop=last)

                for si, (kb, msel) in enumerate(blocks):
                    lo = 0
                    generic_m = None
                    if msel is not None:
                        if msel[0] == "const":
                            lo = KB * msel[1]
                        else:
                            generic_m = msel
                    ksl = slice(kb * KB, (kb + 1) * KB)
                    qlo = slice(qoff + lo, qoff + QSB)
                    st2 = stp.tile([P, 2, QSB], F32, tag="st", name="st2")
                    nc.tensor.matmul(
                        st2[:, 0, lo:QSB], KTb[t][0:DH, ksl],
                        QTb[t][0:DH, qlo],
                        start=True, stop=True, tile_position=(0, 0))
                    nc.tensor.matmul(
                        st2[:, 1, lo:QSB], KTb[t][DH:P, ksl],
                        QTb[t][DH:P, qlo],
                        start=True, stop=True, tile_position=(DH, 0))
                    pt2 = ptpool.tile([P, 2, QSB], BF16, tag="pt", name="pt2")
                    nc.scalar.activation(
                        pt2[:, 0:2, lo:QSB], st2[:, 0:2, lo:QSB], AF.Exp,
                        scale=SCALE)
                    if msel is not None and msel[0] == "const":
                        # ragged 128-col window gets the shared tri-mask
                        # (GpSimd: pt/mk are SBUF, keeps DVE free)
                        w = slice(lo, lo + KB)
                        nc.gpsimd.tensor_tensor(pt2[:, 0, w], pt2[:, 0, w],
                                                mk128[:], OP.mult)
                        nc.gpsimd.tensor_tensor(pt2[:, 1, w], pt2[:, 1, w],
                                                mk128[:], OP.mult)
                    elif generic_m is not None:
                        mg = work.tile([P, QSB], F32, tag="mg")
                        nc.sync.dma_start(mg[:], m01_d[generic_m[1],
                                                       generic_m[2]])
                        mgb = work.tile([P, QSB], BF16, tag="mgb")
                        nc.vector.tensor_copy(mgb[:], mg[:])
                        nc.vector.tensor_tensor(pt2[:, 0, :], pt2[:, 0, :],
                                                mgb[:], OP.mult)
                        nc.vector.tensor_tensor(pt2[:, 1, :], pt2[:, 1, :],
                                                mgb[:], OP.mult)
                    recs.append((pt2, lo, kb))
                    if si >= 1:
                        emit_pv(si - 1)
                    yield
                emit_pv(L - 1)

                # row-sums l live in pv row 64 (V ones column); bf16 copies
                # feed the PE broadcast in the deferred norm
                lrb0 = nrmpool.tile([1, QSB], BF16, tag="lr0", name="lr0")
                lrb1 = nrmpool.tile([1, QSB], BF16, tag="lr1", name="lr1")
                pva, pvb = state["pva"], state["pvb"]
                nc.vector.tensor_copy(lrb0[:], pva[DH:DH + 1, :])
                nc.vector.tensor_copy(lrb1[:], pvb[DH:DH + 1, :])

                def _norm(t=t, qsl=qsl, pva=pva, pvb=pvb, lrb0=lrb0,
                          lrb1=lrb1):
                    # col-tiled pair broadcast of l, then per-head staging
                    # (ACT for h0; DVE partition-shift copy for h1), 1/l, mult
                    bc = spp.tile([P, QSB], F32, tag="sp", name="bc")
                    nc.tensor.matmul(bc[0:DH, :], ones64[:], lrb0[:],
                                     start=True, stop=True,
                                     tile_position=(0, 0))
                    nc.tensor.matmul(bc[DH:P, :], ones64[:], lrb1[:],
                                     start=True, stop=True,
                                     tile_position=(0, DH))
                    bcS0 = work.tile([DH, QSB], F32, tag="bcS",
                                     name="bcS0", bufs=3)
                    bcS1 = work.tile([DH, QSB], F32, tag="bcT",
                                     name="bcS1", bufs=3)
                    nc.scalar.copy(bcS0[:], bc[0:DH, :])
                    nc.vector.tensor_copy(bcS1[:], bc[DH:P, :])
                    for ph, pv, bcS in ((0, pva, bcS0), (DH, pvb, bcS1)):
                        rS = work.tile([DH, QSB], F32, tag="rS",
                                       name="rS", bufs=3)
                        nc.vector.reciprocal_approx_fast(rS[:], bcS[:])
                        nc.vector.tensor_tensor(
                            aT[t][ph:ph + DH, qsl], pv[0:DH, :], rS[:],
                            OP.mult)
                pending_norm.append(_norm)
                yield

            def drain(g):
                for _ in g:
                    pass

            def chain(gens):
                for g in gens:
                    yield from g

            def weave(agen, pgen, ratio):
                """Drain agen; after each yield, advance pgen by `ratio`."""
                acc = 0.0
                alive = True
                for _ in agen:
                    if not alive:
                        continue
                    acc += ratio
                    while acc >= 1.0:
                        if next(pgen, _SENT) is _SENT:
                            alive = False
                            break
                        acc -= 1.0
                for _ in pgen:
                    pass

            _SENT = object()

            # ---------------- interleaved emission ----------------
            # tile 0 projections + all of V up front (V feeds every round,
            # and trace order defines the dependency semantics); grouped by
            # x chunk so early units only wait on early DMA arrivals
            for sc in range(4):
                drain(gen_qk_unit(wqT_d, QTb, 0, sc, "q"))
                drain(gen_qk_unit(wkT_d, KTb, 0, sc, "k"))
                for sb in range(4 * sc, 4 * sc + 4):
                    drain(gen_v_unit(sb))

            # rounds: attention for head pair t woven with tile t+1
            # projections / (round 3) the first half of the output projection,
            # so the PE always has ACT-free matmul work within a HAM window
            n_ayield = sum(len(row) + 1 for row in plan)
            for t in range(4):
                if t == 1:
                    # prefetch all out-proj weights (needed from round 2 on)
                    for oc in range(8):
                        wo = wstream.tile([P, JC, P], BF16, tag="wo", bufs=8,
                                          name=f"wo{oc}")
                        nc.sync.dma_start(wo[:], woT_d[oc])
                        wos.append(wo)
                if t < 3:
                    agen = chain([gen_attn_group(t, qsb)
                                  for qsb in range(NQSB)])
                    pgens = []
                    for sc in range(4):
                        pgens.append(gen_qk_unit(wqT_d, QTb, t + 1, sc, "q"))
                    for sc in range(4):
                        pgens.append(gen_qk_unit(wkT_d, KTb, t + 1, sc, "k"))
                    n_p = 16
                    if t == 2:
                        # outT half (jc 0..2): aT[0]/aT[1] are final now
                        pgens += [gen_op_unit(oc, sc, 0, 2, outT_d)
                                  for oc in range(8) for sc in range(4)]
                        n_p += 32
                    weave(agen, chain(pgens), float(n_p) / n_ayield)
                else:
                    # round 3: outB units (jc 2..4) for q-chunk sc follow
                    # group sc+1 (whose start flushed sc's aT norm)
                    def agen3():
                        for qsb in range(NQSB):
                            yield from gen_attn_group(3, qsb)
                            if qsb >= 1:
                                for oc in range(8):
                                    yield from gen_op_unit(
                                        oc, qsb - 1, 2, JC, outB_d)
                    drain(agen3())

            flush_norm()
            # last outB q-chunk (aT[3] tail)
            for oc in range(8):
                drain(gen_op_unit(oc, 3, 2, JC, outB_d))

    nc.compile()
    return nc


def _plan_key(plan, mode):
    return (mode, tuple(tuple(row) for row in plan))


def _get_compiled(mask):
    plan, mode = _mask_plan(mask)
    key = _plan_key(plan, mode)
    if key not in _COMPILED:
        _COMPILED[key] = (_build(plan, mode), plan, mode)
    return _COMPILED[key]


# --------------------------------------------------------------- host driver

def _make_in_maps(x, Wq, Wk, Wv, Wo, mask, mode):
    cosT2, sinT2, psigT = _host_consts()
    consts = {"cosT": cosT2.astype(ml_dtypes.bfloat16),
              "sinT": sinT2.astype(ml_dtypes.bfloat16),
              "psgT": psigT.astype(ml_dtypes.bfloat16)}
    if mode == "causal":
        # mask128[k, q'] = 1 where q' >= k (ragged diag window, shared)
        consts["mask128"] = np.triu(
            np.ones((P, KB), np.float32)).astype(ml_dtypes.bfloat16)
    elif mode == "generic":
        m = (np.asarray(mask).reshape(S, S) != 0)
        m01 = np.zeros((NQSB, NKB, P, QSB), np.float32)
        for qsb in range(NQSB):
            for kb in range(NKB):
                blk = m[qsb * QSB:(qsb + 1) * QSB, kb * KB:(kb + 1) * KB]
                m01[qsb, kb] = blk.T.astype(np.float32)
        consts["m01"] = m01

    def arr_qk(w):
        # [D, OG_rows] -> per-oc [P, DC, P]: wT[d, o] laid out [oc, p(o), dc, o']
        wT = w.T.astype(np.float32)                       # [D, OG]
        a = wT.reshape(DC, P, 4, P)          # [dc, p(d), oc, o']
        return np.ascontiguousarray(a.transpose(2, 1, 0, 3)).astype(
            ml_dtypes.bfloat16)

    in_maps = []
    for c in range(NCORES):
        b, g = c // HG, c % HG
        rows = slice(OG * g, OG * (g + 1))
        xT = x[b].T.astype(np.float32)                    # [D, S]
        xTa = np.ascontiguousarray(
            xT.reshape(DC, P, NQSB, QSB).transpose(2, 1, 0, 3)).astype(
                ml_dtypes.bfloat16)
        wq = arr_qk(Wq[rows, :])
        wk = arr_qk(Wk[rows, :])
        wvT = np.ascontiguousarray(
            Wv[rows, :].T.astype(np.float32).reshape(DC, P, OG)
            .transpose(1, 0, 2)).astype(ml_dtypes.bfloat16)
        woT = Wo[:, rows].T.astype(np.float32)            # [OG, D]
        woa = np.ascontiguousarray(
            woT.reshape(JC, P, 8, P).transpose(2, 1, 0, 3)
        ).astype(ml_dtypes.bfloat16)
        in_maps.append({
            "xT": xTa,
            "wqT": wq,
            "wkT": wk,
            "wvT": wvT,
            "woT": woa,
            **consts,
        })
    return in_maps


def run(x, Wq, Wk, Wv, Wo, mask, trace=False):
    nc, plan, mode = _get_compiled(mask)
    in_maps = _make_in_maps(x, Wq, Wk, Wv, Wo, mask, mode)
    res = bass_utils.run_bass_kernel_spmd(
        nc, in_maps, core_ids=list(range(NCORES)), trace=trace)
    out = np.empty((B, S, D), np.float32)
    for b in range(B):
        acc = (res.results[2 * b]["outT"].astype(np.float32)
               + res.results[2 * b]["outB"].astype(np.float32)
               + res.results[2 * b + 1]["outT"].astype(np.float32)
               + res.results[2 * b + 1]["outB"].astype(np.float32))
        out[b] = acc.T
    return out, res


def kernel(x, Wq, Wk, Wv, Wo, mask):
    x = np.asarray(x, dtype=np.float32)
    Wq = np.asarray(Wq, dtype=np.float32)
    Wk = np.asarray(Wk, dtype=np.float32)
    Wv = np.asarray(Wv, dtype=np.float32)
    Wo = np.asarray(Wo, dtype=np.float32)
    out, _ = run(x, Wq, Wk, Wv, Wo, mask)
    return out


# revision 47
# speedup vs baseline: 1.0138x; 1.0060x over previous
"""Multi-head attention (RoPE, causal) Trainium2 Bass kernel, 8 NeuronCores.

Problem: x[4,2048,1024] -> MHA(16 heads, head_dim 64, RoPE, causal mask) -> [4,2048,1024]

Sharding (pure data/tensor parallel, no collectives):
  core c -> (batch b = c//2, head-group g = c%2); each head-group = 8 heads = 512 dims.
  Each core computes q/k/v projections for its (batch, head-group), RoPE, attention,
  and a partial output projection (columns of Wo for its head group).
  Host sums the two partial outputs per batch (512-dim contraction split).

Kernel layout tricks:
  - Projections computed in transposed [out_dim, seq] layout (QT/KT) so that
    QK^T blocks come out as S^T [k, q]: softmax reductions along the partition
    dim are avoided entirely via UNSAFE softmax (no row-max; inputs are bounded
    N(0,1)-ish data, logits stay << 88) and the row-sum l is folded into the
    PV matmul by augmenting V with a ones column (M=65).
  - Head-PAIR QK with PE array row tiling: the two heads of a QTb/KTb tile
    live on partitions 0:64 / 64:128, so their K=64 QK matmuls are issued
    back-to-back as row tiles (0,0)/(64,0) and run CONCURRENTLY on the PE
    (~2x).  One merged [128,2,512] psum tile per step feeds a single wide
    EXP, so both banks free together and the pairing survives scheduling.
  - Software pipelining: step si issues QK+exp for block si and PV for block
    si-1, so PV's inputs are always ready and pairs stay adjacent.
  - Causal masking: blocks strictly below the diagonal strip computed full
    width; diagonal-strip blocks compute only columns [lo:512] (saves PE),
    with a single shared [128,128] upper-tri mask multiply on the ragged
    128-col window (GpSimd, off the DVE).  Upper blocks skipped entirely.
  - Normalization: l rows (pv row 64) copied to bf16, broadcast over 64
    partitions via a col-tiled PE matmul pair, reciprocal+multiply per head
    (DVE op count minimized: DVE small-op overhead is ~0.6us each).
  - RoPE via a signed-permutation matrix on the TensorEngine; its two
    SBUF-only elementwise ops run on GpSimd, psum ones on DVE.
  - Out-projection split jc[0:2]->outT (woven into round 2) and
    jc[2:4]->outB (round 3, group-aligned), halved output DMAs on two
    queues; host sums outT+outB per head group.
"""

import numpy as np
import ml_dtypes

import concourse.bass as bass
import concourse.tile as tile
from concourse import bacc, mybir
from concourse import bass_utils

B, S, D, H, DH = 4, 2048, 1024, 16, 64
NCORES = 8
HG = 2              # head groups (tensor parallel)
HPG = H // HG       # heads per group = 8
OG = HPG * DH       # group output dims = 512
SCALE = DH ** -0.5
P = 128
QSB = 512           # q super-block width
NQSB = S // QSB     # 4
KB = 128            # k block
NKB = S // KB       # 16
DC = D // P         # 8 d-chunks
JC = OG // P        # 4 j-chunks (out-proj contraction)

F32 = mybir.dt.float32
F32R = mybir.dt.float32r
BF16 = mybir.dt.bfloat16

_COMPILED = {}


# ---------------------------------------------------------------- host tables

def _rope_tables():
    inv_freq = 1.0 / (10000.0 ** (np.arange(0, DH, 2, dtype=np.float32) / DH))
    t = np.arange(S, dtype=np.float32)
    freqs = np.outer(t, inv_freq).astype(np.float32)      # [S, 32]
    emb = np.concatenate([freqs, freqs], -1)              # [S, 64]
    return np.cos(emb), np.sin(emb)


def _host_consts():
    cos, sin = _rope_tables()                             # [S, 64]
    cosT2 = np.ascontiguousarray(
        np.concatenate([cos.T, cos.T], axis=0), dtype=np.float32)   # [128, S]
    sinT2 = np.ascontiguousarray(
        np.concatenate([sin.T, sin.T], axis=0), dtype=np.float32)
    # signed permutation: rot(x)[i] = -x[i+32] (j<32) else x[i-32], per 64-row head
    psig = np.zeros((P, P), np.float32)
    for i in range(P):
        j = i % DH
        base = (i // DH) * DH
        if j < 32:
            psig[i, base + j + 32] = -1.0
        else:
            psig[i, base + j - 32] = 1.0
    psigT = np.ascontiguousarray(psig.T)
    return cosT2, sinT2, psigT


def _mask_plan(mask):
    """Classify the [S, S] mask into a per-qsb block plan.

    plan[qsb] = list of (kb, msel); msel is None (no mask), ("const", r) for
    the causal diagonal-strip tiles (lo = KB*r), or ("dram", qsb, kb) for
    generic per-block mask tiles.
    """
    m = np.asarray(mask).reshape(S, S) != 0        # [q, k] True = attend
    causal = np.array_equal(m, np.tril(np.ones((S, S), bool)))
    if causal:
        plan = []
        for qsb in range(NQSB):
            row = []
            for kb in range(4 * qsb + 4):
                r = kb - 4 * qsb
                row.append((kb, None if r < 0 else ("const", r)))
            plan.append(row)
        return plan, "causal"
    if m.all():
        return [[(kb, None) for kb in range(NKB)] for _ in range(NQSB)], "full"
    plan = []
    for qsb in range(NQSB):
        row = []
        for kb in range(NKB):
            blk = m[qsb * QSB:(qsb + 1) * QSB, kb * KB:(kb + 1) * KB]  # [q, k]
            if not blk.any():
                continue          # fully masked block contributes nothing
            row.append((kb, None if blk.all() else ("dram", qsb, kb)))
        plan.append(row)
    return plan, "generic"


# ------------------------------------------------------------------- builder

def _build(plan, mode):
    nc = bacc.Bacc("TRN2", target_bir_lowering=False, debug=False, num_devices=1)
    AF = mybir.ActivationFunctionType
    OP = mybir.AluOpType

    xT_d = nc.dram_tensor("xT", [NQSB, P, DC, QSB], BF16,
                          kind="ExternalInput").ap()
    wqT_d = nc.dram_tensor("wqT", [4, P, DC, P], BF16,
                           kind="ExternalInput").ap()
    wkT_d = nc.dram_tensor("wkT", [4, P, DC, P], BF16,
                           kind="ExternalInput").ap()
    wvT_d = nc.dram_tensor("wvT", [P, DC, OG], BF16, kind="ExternalInput").ap()
    woT_d = nc.dram_tensor("woT", [8, P, JC, P], BF16,
                           kind="ExternalInput").ap()
    cos_d = nc.dram_tensor("cosT", [P, S], BF16, kind="ExternalInput").ap()
    sin_d = nc.dram_tensor("sinT", [P, S], BF16, kind="ExternalInput").ap()
    psg_d = nc.dram_tensor("psgT", [P, P], BF16, kind="ExternalInput").ap()
    if mode == "causal":
        mk_d = nc.dram_tensor("mask128", [P, KB], BF16, kind="ExternalInput").ap()
    elif mode == "generic":
        m01_d = nc.dram_tensor("m01", [NQSB, NKB, P, QSB], F32,
                               kind="ExternalInput").ap()
    outT_d = nc.dram_tensor("outT", [D, S], BF16, kind="ExternalOutput").ap()
    outB_d = nc.dram_tensor("outB", [D, S], BF16, kind="ExternalOutput").ap()

    with tile.TileContext(nc) as tc:
        from contextlib import ExitStack
        with ExitStack() as ctx:
            persist = ctx.enter_context(tc.tile_pool(name="persist", bufs=1))
            wstream = ctx.enter_context(tc.tile_pool(name="wstream", bufs=2))
            work = ctx.enter_context(tc.tile_pool(name="work", bufs=2))
            prepool = ctx.enter_context(tc.tile_pool(name="prepool", bufs=2))
            ptpool = ctx.enter_context(tc.tile_pool(name="ptpool", bufs=6))
            nrmpool = ctx.enter_context(tc.tile_pool(name="nrmpool", bufs=2))
            stp = ctx.enter_context(
                tc.tile_pool(name="stp", bufs=2, space="PSUM"))
            spp = ctx.enter_context(
                tc.tile_pool(name="spp", bufs=2, space="PSUM"))
            pvp = ctx.enter_context(
                tc.tile_pool(name="pvp", bufs=2, space="PSUM"))

            # bf16 post-rope Q/K and bf16 V (with ones column) live all-kernel
            QTb = [persist.tile([P, S], BF16, tag=f"qt{t}", name=f"qtb{t}")
                   for t in range(4)]
            KTb = [persist.tile([P, S], BF16, tag=f"kt{t}", name=f"ktb{t}")
                   for t in range(4)]
            V = [persist.tile([P, HPG, DH + 1], BF16, tag=f"v{sb}",
                              name=f"v{sb}") for sb in range(NKB)]
            for sb in range(NKB):
                nc.vector.memset(V[sb][:, :, DH:DH + 1], 1.0)

            # prefetch tile-0 Q/K weight chunks and the first x chunk first
            # on their queues so the first projection matmuls start after a
            # fraction of the ~15 MB bulk DMA
            xTs = [persist.tile([P, DC, QSB], BF16, tag=f"xt{sc}",
                                 name=f"xt{sc}") for sc in range(4)]
            nc.scalar.dma_start(xTs[0][:, 0:4, :], xT_d[0][:, 0:4, :])
            nc.gpsimd.dma_start(xTs[0][:, 4:DC, :], xT_d[0][:, 4:DC, :])
            wqk_live = {}
            for who, w_d in (("q", wqT_d), ("k", wkT_d)):
                w_oc = wstream.tile([P, DC, P], BF16, tag="wqk",
                                    name=f"w{who}0")
                nc.sync.dma_start(w_oc[:, 0:4, :], w_d[0][:, 0:4, :])
                nc.sync.dma_start(w_oc[:, 4:DC, :], w_d[0][:, 4:DC, :])
                wqk_live[who] = w_oc
            psg_sb = persist.tile([P, P], BF16, tag="psg")
            nc.sync.dma_start(psg_sb[:], psg_d)
            wv = persist.tile([P, DC, OG], BF16, tag="wv")
            nc.gpsimd.dma_start(wv[:], wvT_d)
            cos_sb = persist.tile([P, S], BF16, tag="cos")
            sin_sb = persist.tile([P, S], BF16, tag="sin")
            nc.gpsimd.dma_start(cos_sb[:], cos_d)
            nc.gpsimd.dma_start(sin_sb[:], sin_d)
            for sc in range(1, 4):
                nc.sync.dma_start(xTs[sc][:], xT_d[sc])
            aT = [persist.tile([P, S], BF16, tag=f"at{t}", name=f"at{t}")
                  for t in range(4)]
            ones64 = persist.tile([1, DH], BF16, tag="ones64")
            nc.vector.memset(ones64[:], 1.0)
            if mode == "causal":
                mk128 = persist.tile([P, KB], BF16, tag="mk128")
                nc.gpsimd.dma_start(mk128[:], mk_d)

            # ---------------- emitters (generators) ----------------
            # yield points let attention steps and projection halves weave at
            # ~1 us granularity so the PE never sees an ACT-bound stretch

            def gen_qk_unit(w_d, dst, oc, sc, who):
                """One [128, 512] chunk of a Q/K projection + RoPE (2 steps)."""
                if sc == 0 and oc > 0:
                    w_oc = wstream.tile([P, DC, P], BF16, tag="wqk",
                                        name=f"w{who}{oc}")
                    nc.sync.dma_start(w_oc[:], w_d[oc])
                    wqk_live[who] = w_oc
                w_oc = wqk_live[who]
                sl = slice(sc * QSB, (sc + 1) * QSB)
                ps = spp.tile([P, QSB], F32, tag="sp", name="ps")
                for dc in range(4):
                    nc.tensor.matmul(
                        ps[:], w_oc[:, dc, :], xTs[sc][:, dc, :],
                        start=(dc == 0), stop=False)
                yield
                for dc in range(4, DC):
                    nc.tensor.matmul(
                        ps[:], w_oc[:, dc, :], xTs[sc][:, dc, :],
                        start=False, stop=(dc == DC - 1))
                pre = prepool.tile([P, QSB], BF16, tag="pre")
                nc.vector.tensor_copy(pre[:], ps[:])
                rot = spp.tile([P, QSB], F32, tag="sp", name="rot")
                nc.tensor.matmul(rot[:], psg_sb[:], pre[:],
                                 start=True, stop=True)
                m = work.tile([P, QSB], BF16, tag="ropem")
                nc.gpsimd.tensor_tensor(m[:], pre[:], cos_sb[:, sl], OP.mult)
                nc.vector.tensor_tensor(
                    dst[oc][:, sl], rot[:], sin_sb[:, sl], OP.mult)
                nc.gpsimd.tensor_tensor(
                    dst[oc][:, sl], dst[oc][:, sl], m[:], OP.add)
                yield

            def gen_v_unit(sb):
                ps = spp.tile([P, QSB], F32, tag="sp", name="ps")
                xsc, xo = sb // 4, (sb % 4) * P
                for dc in range(4):
                    nc.tensor.matmul(
                        ps[:], xTs[xsc][:, dc, xo:xo + P], wv[:, dc, :],
                        start=(dc == 0), stop=False)
                yield
                for dc in range(4, DC):
                    nc.tensor.matmul(
                        ps[:], xTs[xsc][:, dc, xo:xo + P], wv[:, dc, :],
                        start=False, stop=(dc == DC - 1))
                nc.vector.tensor_copy(
                    V[sb][:, :, 0:DH],
                    ps[:].rearrange("p (h j) -> p h j", j=DH))
                yield

            wos = []

            def gen_op_unit(oc, sc, jlo, jhi, dest):
                """Half of an out-proj psum group (contraction jc in [jlo,jhi))."""
                ssl = slice(sc * QSB, (sc + 1) * QSB)
                ps = spp.tile([P, QSB], F32, tag="sp", name="ps")
                for jc in range(jlo, jhi):
                    nc.tensor.matmul(
                        ps[:], wos[oc][:, jc, :], aT[jc][:, ssl],
                        start=(jc == jlo), stop=(jc == jhi - 1))
                stg = work.tile([P, QSB], BF16, tag="stg", bufs=3, name="stg")
                nc.vector.tensor_copy(stg[:], ps[:])
                e1, e2 = ((nc.sync, nc.scalar), (nc.scalar, nc.sync))[oc % 2]
                half = QSB // 2
                e1.dma_start(dest[oc * P:(oc + 1) * P,
                                  sc * QSB:sc * QSB + half],
                             stg[:, 0:half])
                e2.dma_start(dest[oc * P:(oc + 1) * P,
                                  sc * QSB + half:(sc + 1) * QSB],
                             stg[:, half:QSB])
                yield

            pending_norm = []

            def flush_norm():
                while pending_norm:
                    pending_norm.pop(0)()

            def gen_attn_group(t, qsb):
                """Attention for head pair (2t, 2t+1) on q super-block qsb.

                Software-pipelined: step si issues the row-tiled QK pair and
                exps for block si, then the col-tiled PV pair + 2-way l
                ones-matmuls for block si-1 (whose inputs are all long ready,
                so the scheduler keeps each pair adjacent -> PE concurrency).
                """
                h0, h1 = 2 * t, 2 * t + 1
                qoff = qsb * QSB
                qsl = slice(qoff, qoff + QSB)
                blocks = plan[qsb]
                L = len(blocks)
                state = {}
                recs = []           # si -> (pt0, pt1, lo, kb)

                def emit_pv(s):
                    pt2, lo, kb = recs[s]
                    if s == 0:
                        # run the previous group's deferred normalization
                        # before its pv psum slots are recycled
                        flush_norm()
                        state["pva"] = pvp.tile([DH + 1, QSB], F32,
                                                tag="pv", name="pva")
                        state["pvb"] = pvp.tile([DH + 1, QSB], F32,
                                                tag="pv", name="pvb")
                    first, last = s == 0, s == L - 1
                    nc.tensor.matmul(
                        state["pva"][:, lo:QSB], V[kb][:, h0, :],
                        pt2[:, 0, lo:QSB], start=first, stop=last)
                    nc.tensor.matmul(
                        state["pvb"][:, lo:QSB], V[kb][:, h1, :],
                        pt2[:, 1, lo:QSB], start=first, stop=last)

                for si, (kb, msel) in enumerate(blocks):
                    lo = 0
                    generic_m = None
                    if msel is not None:
                        if msel[0] == "const":
                            lo = KB * msel[1]
                        else:
                            generic_m = msel
                    ksl = slice(kb * KB, (kb + 1) * KB)
                    qlo = slice(qoff + lo, qoff + QSB)
                    st2 = stp.tile([P, 2, QSB], F32, tag="st", name="st2")
                    nc.tensor.matmul(
                        st2[:, 0, lo:QSB], KTb[t][0:DH, ksl],
                        QTb[t][0:DH, qlo],
                        start=True, stop=True, tile_position=(0, 0))
                    nc.tensor.matmul(
                        st2[:, 1, lo:QSB], KTb[t][DH:P, ksl],
                        QTb[t][DH:P, qlo],
                        start=True, stop=True, tile_position=(DH, 0))
                    pt2 = ptpool.tile([P, 2, QSB], BF16, tag="pt", name="pt2")
                    nc.scalar.activation(
                        pt2[:, 0:2, lo:QSB], st2[:, 0:2, lo:QSB], AF.Exp,
                        scale=SCALE)
                    if msel is not None and msel[0] == "const":
                        # ragged 128-col window gets the shared tri-mask
                        # (GpSimd: pt/mk are SBUF, keeps DVE free)
                        w = slice(lo, lo + KB)
                        nc.gpsimd.tensor_tensor(pt2[:, 0, w], pt2[:, 0, w],
                                                mk128[:], OP.mult)
                        nc.gpsimd.tensor_tensor(pt2[:, 1, w], pt2[:, 1, w],
                                                mk128[:], OP.mult)
                    elif generic_m is not None:
                        mg = work.tile([P, QSB], F32, tag="mg")
                        nc.sync.dma_start(mg[:], m01_d[generic_m[1],
                                                       generic_m[2]])
                        mgb = work.tile([P, QSB], BF16, tag="mgb")
                        nc.vector.tensor_copy(mgb[:], mg[:])
                        nc.vector.tensor_tensor(pt2[:, 0, :], pt2[:, 0, :],
                                                mgb[:], OP.mult)
                        nc.vector.tensor_tensor(pt2[:, 1, :], pt2[:, 1, :],
                                                mgb[:], OP.mult)
                    recs.append((pt2, lo, kb))
                    if si >= 1:
                        emit_pv(si - 1)
                    yield
                emit_pv(L - 1)

                # row-sums l live in pv row 64 (V ones column); bf16 copies
                # feed the PE broadcast in the deferred norm
                lrb0 = nrmpool.tile([1, QSB], BF16, tag="lr0", name="lr0")
                lrb1 = nrmpool.tile([1, QSB], BF16, tag="lr1", name="lr1")
                pva, pvb = state["pva"], state["pvb"]
                nc.vector.tensor_copy(lrb0[:], pva[DH:DH + 1, :])
                nc.vector.tensor_copy(lrb1[:], pvb[DH:DH + 1, :])

                def _norm(t=t, qsl=qsl, pva=pva, pvb=pvb, lrb0=lrb0,
                          lrb1=lrb1):
                    # col-tiled pair broadcast of l, then per-head staging
                    # (ACT for h0; DVE partition-shift copy for h1), 1/l, mult
                    bc = spp.tile([P, QSB], F32, tag="sp", name="bc")
                    nc.tensor.matmul(bc[0:DH, :], ones64[:], lrb0[:],
                                     start=True, stop=True,
                                     tile_position=(0, 0))
                    nc.tensor.matmul(bc[DH:P, :], ones64[:], lrb1[:],
                                     start=True, stop=True,
                                     tile_position=(0, DH))
                    bcS0 = work.tile([DH, QSB], F32, tag="bcS",
                                     name="bcS0", bufs=3)
                    bcS1 = work.tile([DH, QSB], F32, tag="bcT",
                                     name="bcS1", bufs=3)
                    nc.scalar.copy(bcS0[:], bc[0:DH, :])
                    nc.vector.tensor_copy(bcS1[:], bc[DH:P, :])
                    for ph, pv, bcS in ((0, pva, bcS0), (DH, pvb, bcS1)):
                        rS = work.tile([DH, QSB], F32, tag="rS",
                                       name="rS", bufs=3)
                        nc.vector.reciprocal_approx_fast(rS[:], bcS[:])
                        nc.vector.tensor_tensor(
                            aT[t][ph:ph + DH, qsl], pv[0:DH, :], rS[:],
                            OP.mult)
                pending_norm.append(_norm)
                yield

            def drain(g):
                for _ in g:
                    pass

            def chain(gens):
                for g in gens:
                    yield from g

            def weave(agen, pgen, ratio):
                """Drain agen; after each yield, advance pgen by `ratio`."""
                acc = 0.0
                alive = True
                for _ in agen:
                    if not alive:
                        continue
                    acc += ratio
                    while acc >= 1.0:
                        if next(pgen, _SENT) is _SENT:
                            alive = False
                            break
                        acc -= 1.0
                for _ in pgen:
                    pass

            _SENT = object()

            # ---------------- interleaved emission ----------------
            # tile 0 projections + all of V up front (V feeds every round,
            # and trace order defines the dependency semantics); grouped by
            # x chunk so early units only wait on early DMA arrivals
            for sc in range(4):
                drain(gen_qk_unit(wqT_d, QTb, 0, sc, "q"))
                drain(gen_qk_unit(wkT_d, KTb, 0, sc, "k"))
                for sb in range(4 * sc, 4 * sc + 4):
                    drain(gen_v_unit(sb))

            # rounds: attention for head pair t woven with tile t+1
            # projections / (round 3) the first half of the output projection,
            # so the PE always has ACT-free matmul work within a HAM window
            n_ayield = sum(len(row) + 1 for row in plan)
            for t in range(4):
                if t == 1:
                    # prefetch all out-proj weights (needed from round 2 on)
                    for oc in range(8):
                        wo = wstream.tile([P, JC, P], BF16, tag="wo", bufs=8,
                                          name=f"wo{oc}")
                        nc.sync.dma_start(wo[:], woT_d[oc])
                        wos.append(wo)
                if t < 3:
                    agen = chain([gen_attn_group(t, qsb)
                                  for qsb in range(NQSB)])
                    pgens = []
                    for sc in range(4):
                        pgens.append(gen_qk_unit(wqT_d, QTb, t + 1, sc, "q"))
                    for sc in range(4):
                        pgens.append(gen_qk_unit(wkT_d, KTb, t + 1, sc, "k"))
                    n_p = 16
                    if t == 2:
                        # outT half (jc 0..2): aT[0]/aT[1] are final now
                        pgens += [gen_op_unit(oc, sc, 0, 2, outT_d)
                                  for oc in range(8) for sc in range(4)]
                        n_p += 32
                    weave(agen, chain(pgens), float(n_p) / n_ayield)
                else:
                    # round 3: outB units (jc 2..4) for q-chunk sc follow
                    # group sc+1 (whose start flushed sc's aT norm)
                    def agen3():
                        for qsb in range(NQSB):
                            yield from gen_attn_group(3, qsb)
                            if qsb >= 1:
                                for oc in range(8):
                                    yield from gen_op_unit(
                                        oc, qsb - 1, 2, JC, outB_d)
                    drain(agen3())

            flush_norm()
            # last outB q-chunk (aT[3] tail)
            for oc in range(8):
                drain(gen_op_unit(oc, 3, 2, JC, outB_d))

    nc.compile()
    return nc


def _plan_key(plan, mode):
    return (mode, tuple(tuple(row) for row in plan))


def _get_compiled(mask):
    plan, mode = _mask_plan(mask)
    key = _plan_key(plan, mode)
    if key not in _COMPILED:
        _COMPILED[key] = (_build(plan, mode), plan, mode)
    return _COMPILED[key]


# --------------------------------------------------------------- host driver

def _make_in_maps(x, Wq, Wk, Wv, Wo, mask, mode):
    cosT2, sinT2, psigT = _host_consts()
    consts = {"cosT": cosT2.astype(ml_dtypes.bfloat16),
              "sinT": sinT2.astype(ml_dtypes.bfloat16),
              "psgT": psigT.astype(ml_dtypes.bfloat16)}
    if mode == "causal":
        # mask128[k, q'] = 1 where q' >= k (ragged diag window, shared)
        consts["mask128"] = np.triu(
            np.ones((P, KB), np.float32)).astype(ml_dtypes.bfloat16)
    elif mode == "generic":
        m = (np.asarray(mask).reshape(S, S) != 0)
        m01 = np.zeros((NQSB, NKB, P, QSB), np.float32)
        for qsb in range(NQSB):
            for kb in range(NKB):
                blk = m[qsb * QSB:(qsb + 1) * QSB, kb * KB:(kb + 1) * KB]
                m01[qsb, kb] = blk.T.astype(np.float32)
        consts["m01"] = m01

    def arr_qk(w):
        # [D, OG_rows] -> per-oc [P, DC, P]: wT[d, o] laid out [oc, p(o), dc, o']
        wT = w.T.astype(np.float32)                       # [D, OG]
        a = wT.reshape(DC, P, 4, P)          # [dc, p(d), oc, o']
        return np.ascontiguousarray(a.transpose(2, 1, 0, 3)).astype(
            ml_dtypes.bfloat16)

    in_maps = []
    for c in range(NCORES):
        b, g = c // HG, c % HG
        rows = slice(OG * g, OG * (g + 1))
        xT = x[b].T.astype(np.float32)                    # [D, S]
        xTa = np.ascontiguousarray(
            xT.reshape(DC, P, NQSB, QSB).transpose(2, 1, 0, 3)).astype(
                ml_dtypes.bfloat16)
        wq = arr_qk(Wq[rows, :])
        wk = arr_qk(Wk[rows, :])
        wvT = np.ascontiguousarray(
            Wv[rows, :].T.astype(np.float32).reshape(DC, P, OG)
            .transpose(1, 0, 2)).astype(ml_dtypes.bfloat16)
        woT = Wo[:, rows].T.astype(np.float32)            # [OG, D]
        woa = np.ascontiguousarray(
            woT.reshape(JC, P, 8, P).transpose(2, 1, 0, 3)
        ).astype(ml_dtypes.bfloat16)
        in_maps.append({
            "xT": xTa,
            "wqT": wq,
            "wkT": wk,
            "wvT": wvT,
            "woT": woa,
            **consts,
        })
    return in_maps


def run(x, Wq, Wk, Wv, Wo, mask, trace=False):
    nc, plan, mode = _get_compiled(mask)
    in_maps = _make_in_maps(x, Wq, Wk, Wv, Wo, mask, mode)
    res = bass_utils.run_bass_kernel_spmd(
        nc, in_maps, core_ids=list(range(NCORES)), trace=trace)
    out = np.empty((B, S, D), np.float32)
    for b in range(B):
        acc = (res.results[2 * b]["outT"].astype(np.float32)
               + res.results[2 * b]["outB"].astype(np.float32)
               + res.results[2 * b + 1]["outT"].astype(np.float32)
               + res.results[2 * b + 1]["outB"].astype(np.float32))
        out[b] = acc.T
    return out, res


def kernel(x, Wq, Wk, Wv, Wo, mask):
    x = np.asarray(x, dtype=np.float32)
    Wq = np.asarray(Wq, dtype=np.float32)
    Wk = np.asarray(Wk, dtype=np.float32)
    Wv = np.asarray(Wv, dtype=np.float32)
    Wo = np.asarray(Wo, dtype=np.float32)
    out, _ = run(x, Wq, Wk, Wv, Wo, mask)
    return out


# revision 48
# speedup vs baseline: 1.0469x; 1.0326x over previous
"""Multi-head attention (RoPE, causal) Trainium2 Bass kernel, 8 NeuronCores.

Problem: x[4,2048,1024] -> MHA(16 heads, head_dim 64, RoPE, causal mask) -> [4,2048,1024]

Sharding (pure data/tensor parallel, no collectives):
  core c -> (batch b = c//2, head-group g = c%2); each head-group = 8 heads = 512 dims.
  Each core computes q/k/v projections for its (batch, head-group), RoPE, attention,
  and a partial output projection (columns of Wo for its head group).
  Host sums the two partial outputs per batch (512-dim contraction split).

Kernel layout tricks:
  - Projections computed in transposed [out_dim, seq] layout (QT/KT) so that
    QK^T blocks come out as S^T [k, q]: softmax reductions along the partition
    dim are avoided entirely via UNSAFE softmax (no row-max; inputs are bounded
    N(0,1)-ish data, logits stay << 88) and the row-sum l is folded into the
    PV matmul by augmenting V with a ones column (M=65).
  - Head-PAIR QK with PE array row tiling: the two heads of a QTb/KTb tile
    live on partitions 0:64 / 64:128, so their K=64 QK matmuls are issued
    back-to-back as row tiles (0,0)/(64,0) and run CONCURRENTLY on the PE
    (~2x).  One merged [128,2,512] psum tile per step feeds a single wide
    EXP, so both banks free together and the pairing survives scheduling.
  - Software pipelining: step si issues QK+exp for block si and PV for block
    si-1, so PV's inputs are always ready and pairs stay adjacent.
  - Causal masking: blocks strictly below the diagonal strip computed full
    width; diagonal-strip blocks compute only columns [lo:512] (saves PE),
    with a single shared [128,128] upper-tri mask multiply on the ragged
    128-col window (GpSimd, off the DVE).  Upper blocks skipped entirely.
  - Normalization: l rows (pv row 64) copied to bf16, broadcast over 64
    partitions via a col-tiled PE matmul pair, reciprocal+multiply per head
    (DVE op count minimized: DVE small-op overhead is ~0.6us each).
  - RoPE via a signed-permutation matrix on the TensorEngine; its two
    SBUF-only elementwise ops run on GpSimd, psum ones on DVE.
  - Out-projection split jc[0:2]->outT (woven into round 2) and
    jc[2:4]->outB (round 3, group-aligned), halved output DMAs on two
    queues; host sums outT+outB per head group.
"""

import numpy as np
import ml_dtypes

import concourse.bass as bass
import concourse.tile as tile
from concourse import bacc, mybir
from concourse import bass_utils

B, S, D, H, DH = 4, 2048, 1024, 16, 64
NCORES = 8
HG = 2              # head groups (tensor parallel)
HPG = H // HG       # heads per group = 8
OG = HPG * DH       # group output dims = 512
SCALE = DH ** -0.5
P = 128
QSB = 512           # q super-block width
NQSB = S // QSB     # 4
KB = 128            # k block
NKB = S // KB       # 16
DC = D // P         # 8 d-chunks
JC = OG // P        # 4 j-chunks (out-proj contraction)

F32 = mybir.dt.float32
F32R = mybir.dt.float32r
BF16 = mybir.dt.bfloat16

_COMPILED = {}


# ---------------------------------------------------------------- host tables

def _rope_tables():
    inv_freq = 1.0 / (10000.0 ** (np.arange(0, DH, 2, dtype=np.float32) / DH))
    t = np.arange(S, dtype=np.float32)
    freqs = np.outer(t, inv_freq).astype(np.float32)      # [S, 32]
    emb = np.concatenate([freqs, freqs], -1)              # [S, 64]
    return np.cos(emb), np.sin(emb)


def _host_consts():
    cos, sin = _rope_tables()                             # [S, 64]
    cosT2 = np.ascontiguousarray(
        np.concatenate([cos.T, cos.T], axis=0), dtype=np.float32)   # [128, S]
    sinT2 = np.ascontiguousarray(
        np.concatenate([sin.T, sin.T], axis=0), dtype=np.float32)
    # signed permutation: rot(x)[i] = -x[i+32] (j<32) else x[i-32], per 64-row head
    psig = np.zeros((P, P), np.float32)
    for i in range(P):
        j = i % DH
        base = (i // DH) * DH
        if j < 32:
            psig[i, base + j + 32] = -1.0
        else:
            psig[i, base + j - 32] = 1.0
    psigT = np.ascontiguousarray(psig.T)
    return cosT2, sinT2, psigT


def _mask_plan(mask):
    """Classify the [S, S] mask into a per-qsb block plan.

    plan[qsb] = list of (kb, msel); msel is None (no mask), ("const", r) for
    the causal diagonal-strip tiles (lo = KB*r), or ("dram", qsb, kb) for
    generic per-block mask tiles.
    """
    m = np.asarray(mask).reshape(S, S) != 0        # [q, k] True = attend
    causal = np.array_equal(m, np.tril(np.ones((S, S), bool)))
    if causal:
        plan = []
        for qsb in range(NQSB):
            row = []
            for kb in range(4 * qsb + 4):
                r = kb - 4 * qsb
                row.append((kb, None if r < 0 else ("const", r)))
            plan.append(row)
        return plan, "causal"
    if m.all():
        return [[(kb, None) for kb in range(NKB)] for _ in range(NQSB)], "full"
    plan = []
    for qsb in range(NQSB):
        row = []
        for kb in range(NKB):
            blk = m[qsb * QSB:(qsb + 1) * QSB, kb * KB:(kb + 1) * KB]  # [q, k]
            if not blk.any():
                continue          # fully masked block contributes nothing
            row.append((kb, None if blk.all() else ("dram", qsb, kb)))
        plan.append(row)
    return plan, "generic"


# ------------------------------------------------------------------- builder

def _build(plan, mode):
    nc = bacc.Bacc("TRN2", target_bir_lowering=False, debug=False, num_devices=1)
    AF = mybir.ActivationFunctionType
    OP = mybir.AluOpType

    xT_d = nc.dram_tensor("xT", [NQSB, P, DC, QSB], BF16,
                          kind="ExternalInput").ap()
    wqT_d = nc.dram_tensor("wqT", [4, P, DC, P], BF16,
                           kind="ExternalInput").ap()
    wkT_d = nc.dram_tensor("wkT", [4, P, DC, P], BF16,
                           kind="ExternalInput").ap()
    wvT_d = nc.dram_tensor("wvT", [P, DC, OG], BF16, kind="ExternalInput").ap()
    woT_d = nc.dram_tensor("woT", [8, P, JC, P], BF16,
                           kind="ExternalInput").ap()
    cos_d = nc.dram_tensor("cosT", [P, S], BF16, kind="ExternalInput").ap()
    sin_d = nc.dram_tensor("sinT", [P, S], BF16, kind="ExternalInput").ap()
    psg_d = nc.dram_tensor("psgT", [P, P], BF16, kind="ExternalInput").ap()
    if mode == "causal":
        mk_d = nc.dram_tensor("mask128", [P, KB], BF16, kind="ExternalInput").ap()
    elif mode == "generic":
        m01_d = nc.dram_tensor("m01", [NQSB, NKB, P, QSB], F32,
                               kind="ExternalInput").ap()
    outT_d = nc.dram_tensor("outT", [D, S], BF16, kind="ExternalOutput").ap()
    outB_d = nc.dram_tensor("outB", [D, S], BF16, kind="ExternalOutput").ap()

    with tile.TileContext(nc) as tc:
        from contextlib import ExitStack
        with ExitStack() as ctx:
            persist = ctx.enter_context(tc.tile_pool(name="persist", bufs=1))
            wstream = ctx.enter_context(tc.tile_pool(name="wstream", bufs=2))
            work = ctx.enter_context(tc.tile_pool(name="work", bufs=2))
            prepool = ctx.enter_context(tc.tile_pool(name="prepool", bufs=4))
            ptpool = ctx.enter_context(tc.tile_pool(name="ptpool", bufs=8))
            nrmpool = ctx.enter_context(tc.tile_pool(name="nrmpool", bufs=2))
            stp = ctx.enter_context(
                tc.tile_pool(name="stp", bufs=2, space="PSUM"))
            spp = ctx.enter_context(
                tc.tile_pool(name="spp", bufs=2, space="PSUM"))
            pvp = ctx.enter_context(
                tc.tile_pool(name="pvp", bufs=2, space="PSUM"))

            # bf16 post-rope Q/K and bf16 V (with ones column) live all-kernel
            QTb = [persist.tile([P, S], BF16, tag=f"qt{t}", name=f"qtb{t}")
                   for t in range(4)]
            KTb = [persist.tile([P, S], BF16, tag=f"kt{t}", name=f"ktb{t}")
                   for t in range(4)]
            V = [persist.tile([P, HPG, DH + 1], BF16, tag=f"v{sb}",
                              name=f"v{sb}") for sb in range(NKB)]
            for sb in range(NKB):
                nc.vector.memset(V[sb][:, :, DH:DH + 1], 1.0)

            # prefetch tile-0 Q/K weight chunks and the first x chunk first
            # on their queues so the first projection matmuls start after a
            # fraction of the ~15 MB bulk DMA
            xTs = [persist.tile([P, DC, QSB], BF16, tag=f"xt{sc}",
                                 name=f"xt{sc}") for sc in range(4)]
            nc.scalar.dma_start(xTs[0][:, 0:4, :], xT_d[0][:, 0:4, :])
            nc.gpsimd.dma_start(xTs[0][:, 4:DC, :], xT_d[0][:, 4:DC, :])
            wqk_live = {}
            for who, w_d in (("q", wqT_d), ("k", wkT_d)):
                w_oc = wstream.tile([P, DC, P], BF16, tag="wqk",
                                    name=f"w{who}0")
                nc.sync.dma_start(w_oc[:, 0:4, :], w_d[0][:, 0:4, :])
                nc.sync.dma_start(w_oc[:, 4:DC, :], w_d[0][:, 4:DC, :])
                wqk_live[who] = w_oc
            psg_sb = persist.tile([P, P], BF16, tag="psg")
            nc.sync.dma_start(psg_sb[:], psg_d)
            wv = persist.tile([P, DC, OG], BF16, tag="wv")
            nc.gpsimd.dma_start(wv[:], wvT_d)
            cos_sb = persist.tile([P, S], BF16, tag="cos")
            sin_sb = persist.tile([P, S], BF16, tag="sin")
            nc.gpsimd.dma_start(cos_sb[:], cos_d)
            nc.gpsimd.dma_start(sin_sb[:], sin_d)
            for sc in range(1, 4):
                nc.sync.dma_start(xTs[sc][:], xT_d[sc])
            aT = [persist.tile([P, S], BF16, tag=f"at{t}", name=f"at{t}")
                  for t in range(4)]
            ones64 = persist.tile([1, DH], BF16, tag="ones64")
            nc.vector.memset(ones64[:], 1.0)
            if mode == "causal":
                mk128 = persist.tile([P, KB], BF16, tag="mk128")
                nc.gpsimd.dma_start(mk128[:], mk_d)

            # ---------------- emitters (generators) ----------------
            # yield points let attention steps and projection halves weave at
            # ~1 us granularity so the PE never sees an ACT-bound stretch

            def gen_qk_unit(w_d, dst, oc, sc, who):
                """One [128, 512] chunk of a Q/K projection + RoPE (2 steps)."""
                if sc == 0 and oc > 0:
                    w_oc = wstream.tile([P, DC, P], BF16, tag="wqk",
                                        name=f"w{who}{oc}")
                    nc.sync.dma_start(w_oc[:], w_d[oc])
                    wqk_live[who] = w_oc
                w_oc = wqk_live[who]
                sl = slice(sc * QSB, (sc + 1) * QSB)
                ps = spp.tile([P, QSB], F32, tag="sp", name="ps")
                for dc in range(4):
                    nc.tensor.matmul(
                        ps[:], w_oc[:, dc, :], xTs[sc][:, dc, :],
                        start=(dc == 0), stop=False)
                yield
                for dc in range(4, DC):
                    nc.tensor.matmul(
                        ps[:], w_oc[:, dc, :], xTs[sc][:, dc, :],
                        start=False, stop=(dc == DC - 1))
                pre = prepool.tile([P, QSB], BF16, tag="pre")
                nc.vector.tensor_copy(pre[:], ps[:])
                rot = spp.tile([P, QSB], F32, tag="sp", name="rot")
                nc.tensor.matmul(rot[:], psg_sb[:], pre[:],
                                 start=True, stop=True)
                m = work.tile([P, QSB], BF16, tag="ropem")
                nc.gpsimd.tensor_tensor(m[:], pre[:], cos_sb[:, sl], OP.mult)
                nc.vector.tensor_tensor(
                    dst[oc][:, sl], rot[:], sin_sb[:, sl], OP.mult)
                nc.gpsimd.tensor_tensor(
                    dst[oc][:, sl], dst[oc][:, sl], m[:], OP.add)
                yield

            def gen_v_unit(sb):
                ps = spp.tile([P, QSB], F32, tag="sp", name="ps")
                xsc, xo = sb // 4, (sb % 4) * P
                for dc in range(4):
                    nc.tensor.matmul(
                        ps[:], xTs[xsc][:, dc, xo:xo + P], wv[:, dc, :],
                        start=(dc == 0), stop=False)
                yield
                for dc in range(4, DC):
                    nc.tensor.matmul(
                        ps[:], xTs[xsc][:, dc, xo:xo + P], wv[:, dc, :],
                        start=False, stop=(dc == DC - 1))
                nc.vector.tensor_copy(
                    V[sb][:, :, 0:DH],
                    ps[:].rearrange("p (h j) -> p h j", j=DH))
                yield

            wos = []

            def gen_op_unit(oc, sc, jlo, jhi, dest):
                """Half of an out-proj psum group (contraction jc in [jlo,jhi))."""
                ssl = slice(sc * QSB, (sc + 1) * QSB)
                ps = spp.tile([P, QSB], F32, tag="sp", name="ps")
                for jc in range(jlo, jhi):
                    nc.tensor.matmul(
                        ps[:], wos[oc][:, jc, :], aT[jc][:, ssl],
                        start=(jc == jlo), stop=(jc == jhi - 1))
                stg = work.tile([P, QSB], BF16, tag="stg", bufs=4, name="stg")
                nc.vector.tensor_copy(stg[:], ps[:])
                e1, e2 = ((nc.sync, nc.scalar), (nc.scalar, nc.sync))[oc % 2]
                half = QSB // 2
                e1.dma_start(dest[oc * P:(oc + 1) * P,
                                  sc * QSB:sc * QSB + half],
                             stg[:, 0:half])
                e2.dma_start(dest[oc * P:(oc + 1) * P,
                                  sc * QSB + half:(sc + 1) * QSB],
                             stg[:, half:QSB])
                yield

            pending_norm = []

            def flush_norm():
                while pending_norm:
                    pending_norm.pop(0)()

            def gen_attn_group(t, qsb):
                """Attention for head pair (2t, 2t+1) on q super-block qsb.

                Software-pipelined: step si issues the row-tiled QK pair and
                exps for block si, then the col-tiled PV pair + 2-way l
                ones-matmuls for block si-1 (whose inputs are all long ready,
                so the scheduler keeps each pair adjacent -> PE concurrency).
                """
                h0, h1 = 2 * t, 2 * t + 1
                qoff = qsb * QSB
                qsl = slice(qoff, qoff + QSB)
                blocks = plan[qsb]
                L = len(blocks)
                state = {}
                recs = []           # si -> (pt0, pt1, lo, kb)

                def emit_pv(s):
                    pt2, lo, kb = recs[s]
                    if s == 0:
                        # run the previous group's deferred normalization
                        # before its pv psum slots are recycled
                        flush_norm()
                        state["pva"] = pvp.tile([DH + 1, QSB], F32,
                                                tag="pv", name="pva")
                        state["pvb"] = pvp.tile([DH + 1, QSB], F32,
                                                tag="pv", name="pvb")
                    first, last = s == 0, s == L - 1
                    nc.tensor.matmul(
                        state["pva"][:, lo:QSB], V[kb][:, h0, :],
                        pt2[:, 0, lo:QSB], start=first, stop=last)
                    nc.tensor.matmul(
                        state["pvb"][:, lo:QSB], V[kb][:, h1, :],
                        pt2[:, 1, lo:QSB], start=first, stop=last)

                for si, (kb, msel) in enumerate(blocks):
                    lo = 0
                    generic_m = None
                    if msel is not None:
                        if msel[0] == "const":
                            lo = KB * msel[1]
                        else:
                            generic_m = msel
                    ksl = slice(kb * KB, (kb + 1) * KB)
                    qlo = slice(qoff + lo, qoff + QSB)
                    st2 = stp.tile([P, 2, QSB], F32, tag="st", name="st2")
                    nc.tensor.matmul(
                        st2[:, 0, lo:QSB], KTb[t][0:DH, ksl],
                        QTb[t][0:DH, qlo],
                        start=True, stop=True, tile_position=(0, 0))
                    nc.tensor.matmul(
                        st2[:, 1, lo:QSB], KTb[t][DH:P, ksl],
                        QTb[t][DH:P, qlo],
                        start=True, stop=True, tile_position=(DH, 0))
                    pt2 = ptpool.tile([P, 2, QSB], BF16, tag="pt", name="pt2")
                    nc.scalar.activation(
                        pt2[:, 0:2, lo:QSB], st2[:, 0:2, lo:QSB], AF.Exp,
                        scale=SCALE)
                    if msel is not None and msel[0] == "const":
                        # ragged 128-col window gets the shared tri-mask
                        # (GpSimd: pt/mk are SBUF, keeps DVE free)
                        w = slice(lo, lo + KB)
                        nc.gpsimd.tensor_tensor(pt2[:, 0, w], pt2[:, 0, w],
                                                mk128[:], OP.mult)
                        nc.gpsimd.tensor_tensor(pt2[:, 1, w], pt2[:, 1, w],
                                                mk128[:], OP.mult)
                    elif generic_m is not None:
                        mg = work.tile([P, QSB], F32, tag="mg")
                        nc.sync.dma_start(mg[:], m01_d[generic_m[1],
                                                       generic_m[2]])
                        mgb = work.tile([P, QSB], BF16, tag="mgb")
                        nc.vector.tensor_copy(mgb[:], mg[:])
                        nc.vector.tensor_tensor(pt2[:, 0, :], pt2[:, 0, :],
                                                mgb[:], OP.mult)
                        nc.vector.tensor_tensor(pt2[:, 1, :], pt2[:, 1, :],
                                                mgb[:], OP.mult)
                    recs.append((pt2, lo, kb))
                    if si >= 1:
                        emit_pv(si - 1)
                    yield
                emit_pv(L - 1)

                # row-sums l live in pv row 64 (V ones column); bf16 copies
                # feed the PE broadcast in the deferred norm
                lrb0 = nrmpool.tile([1, QSB], BF16, tag="lr0", name="lr0")
                lrb1 = nrmpool.tile([1, QSB], BF16, tag="lr1", name="lr1")
                pva, pvb = state["pva"], state["pvb"]
                nc.vector.tensor_copy(lrb0[:], pva[DH:DH + 1, :])
                nc.vector.tensor_copy(lrb1[:], pvb[DH:DH + 1, :])

                def _norm(t=t, qsl=qsl, pva=pva, pvb=pvb, lrb0=lrb0,
                          lrb1=lrb1):
                    # col-tiled pair broadcast of l, then per-head staging
                    # (ACT for h0; DVE partition-shift copy for h1), 1/l, mult
                    bc = spp.tile([P, QSB], F32, tag="sp", name="bc")
                    nc.tensor.matmul(bc[0:DH, :], ones64[:], lrb0[:],
                                     start=True, stop=True,
                                     tile_position=(0, 0))
                    nc.tensor.matmul(bc[DH:P, :], ones64[:], lrb1[:],
                                     start=True, stop=True,
                                     tile_position=(0, DH))
                    bcS0 = work.tile([DH, QSB], F32, tag="bcS",
                                     name="bcS0", bufs=3)
                    bcS1 = work.tile([DH, QSB], F32, tag="bcT",
                                     name="bcS1", bufs=3)
                    nc.scalar.copy(bcS0[:], bc[0:DH, :])
                    nc.vector.tensor_copy(bcS1[:], bc[DH:P, :])
                    for ph, pv, bcS in ((0, pva, bcS0), (DH, pvb, bcS1)):
                        rS = work.tile([DH, QSB], F32, tag="rS",
                                       name="rS", bufs=3)
                        nc.vector.reciprocal_approx_fast(rS[:], bcS[:])
                        nc.vector.tensor_tensor(
                            aT[t][ph:ph + DH, qsl], pv[0:DH, :], rS[:],
                            OP.mult)
                pending_norm.append(_norm)
                yield

            def drain(g):
                for _ in g:
                    pass

            def chain(gens):
                for g in gens:
                    yield from g

            def weave(agen, pgen, ratio):
                """Drain agen; after each yield, advance pgen by `ratio`."""
                acc = 0.0
                alive = True
                for _ in agen:
                    if not alive:
                        continue
                    acc += ratio
                    while acc >= 1.0:
                        if next(pgen, _SENT) is _SENT:
                            alive = False
                            break
                        acc -= 1.0
                for _ in pgen:
                    pass

            _SENT = object()

            # ---------------- interleaved emission ----------------
            # tile 0 projections + all of V up front (V feeds every round,
            # and trace order defines the dependency semantics); grouped by
            # x chunk so early units only wait on early DMA arrivals
            for sc in range(4):
                drain(gen_qk_unit(wqT_d, QTb, 0, sc, "q"))
                drain(gen_qk_unit(wkT_d, KTb, 0, sc, "k"))
                for sb in range(4 * sc, 4 * sc + 4):
                    drain(gen_v_unit(sb))

            # rounds: attention for head pair t woven with tile t+1
            # projections / (round 3) the first half of the output projection,
            # so the PE always has ACT-free matmul work within a HAM window
            n_ayield = sum(len(row) + 1 for row in plan)
            for t in range(4):
                if t == 1:
                    # prefetch all out-proj weights (needed from round 2 on)
                    for oc in range(8):
                        wo = wstream.tile([P, JC, P], BF16, tag="wo", bufs=8,
                                          name=f"wo{oc}")
                        nc.scalar.dma_start(wo[:], woT_d[oc])
                        wos.append(wo)
                if t < 3:
                    agen = chain([gen_attn_group(t, qsb)
                                  for qsb in range(NQSB)])
                    pgens = []
                    for sc in range(4):
                        pgens.append(gen_qk_unit(wqT_d, QTb, t + 1, sc, "q"))
                    for sc in range(4):
                        pgens.append(gen_qk_unit(wkT_d, KTb, t + 1, sc, "k"))
                    n_p = 16
                    if t == 2:
                        # outT half (jc 0..2): aT[0]/aT[1] are final now
                        pgens += [gen_op_unit(oc, sc, 0, 2, outT_d)
                                  for oc in range(8) for sc in range(4)]
                        n_p += 32
                    weave(agen, chain(pgens), float(n_p) / n_ayield)
                else:
                    # round 3: outB units (jc 2..4) for q-chunk sc follow
                    # group sc+1 (whose start flushed sc's aT norm)
                    def agen3():
                        for qsb in range(NQSB):
                            yield from gen_attn_group(3, qsb)
                            if qsb >= 1:
                                for oc in range(8):
                                    yield from gen_op_unit(
                                        oc, qsb - 1, 2, JC, outB_d)
                    drain(agen3())

            flush_norm()
            # last outB q-chunk (aT[3] tail)
            for oc in range(8):
                drain(gen_op_unit(oc, 3, 2, JC, outB_d))

    nc.compile()
    return nc


def _plan_key(plan, mode):
    return (mode, tuple(tuple(row) for row in plan))


def _get_compiled(mask):
    plan, mode = _mask_plan(mask)
    key = _plan_key(plan, mode)
    if key not in _COMPILED:
        _COMPILED[key] = (_build(plan, mode), plan, mode)
    return _COMPILED[key]


# --------------------------------------------------------------- host driver

def _make_in_maps(x, Wq, Wk, Wv, Wo, mask, mode):
    cosT2, sinT2, psigT = _host_consts()
    consts = {"cosT": cosT2.astype(ml_dtypes.bfloat16),
              "sinT": sinT2.astype(ml_dtypes.bfloat16),
              "psgT": psigT.astype(ml_dtypes.bfloat16)}
    if mode == "causal":
        # mask128[k, q'] = 1 where q' >= k (ragged diag window, shared)
        consts["mask128"] = np.triu(
            np.ones((P, KB), np.float32)).astype(ml_dtypes.bfloat16)
    elif mode == "generic":
        m = (np.asarray(mask).reshape(S, S) != 0)
        m01 = np.zeros((NQSB, NKB, P, QSB), np.float32)
        for qsb in range(NQSB):
            for kb in range(NKB):
                blk = m[qsb * QSB:(qsb + 1) * QSB, kb * KB:(kb + 1) * KB]
                m01[qsb, kb] = blk.T.astype(np.float32)
        consts["m01"] = m01

    def arr_qk(w):
        # [D, OG_rows] -> per-oc [P, DC, P]: wT[d, o] laid out [oc, p(o), dc, o']
        wT = w.T.astype(np.float32)                       # [D, OG]
        a = wT.reshape(DC, P, 4, P)          # [dc, p(d), oc, o']
        return np.ascontiguousarray(a.transpose(2, 1, 0, 3)).astype(
            ml_dtypes.bfloat16)

    in_maps = []
    for c in range(NCORES):
        b, g = c // HG, c % HG
        rows = slice(OG * g, OG * (g + 1))
        xT = x[b].T.astype(np.float32)                    # [D, S]
        xTa = np.ascontiguousarray(
            xT.reshape(DC, P, NQSB, QSB).transpose(2, 1, 0, 3)).astype(
                ml_dtypes.bfloat16)
        wq = arr_qk(Wq[rows, :])
        wk = arr_qk(Wk[rows, :])
        wvT = np.ascontiguousarray(
            Wv[rows, :].T.astype(np.float32).reshape(DC, P, OG)
            .transpose(1, 0, 2)).astype(ml_dtypes.bfloat16)
        woT = Wo[:, rows].T.astype(np.float32)            # [OG, D]
        woa = np.ascontiguousarray(
            woT.reshape(JC, P, 8, P).transpose(2, 1, 0, 3)
        ).astype(ml_dtypes.bfloat16)
        in_maps.append({
            "xT": xTa,
            "wqT": wq,
            "wkT": wk,
            "wvT": wvT,
            "woT": woa,
            **consts,
        })
    return in_maps


def run(x, Wq, Wk, Wv, Wo, mask, trace=False):
    nc, plan, mode = _get_compiled(mask)
    in_maps = _make_in_maps(x, Wq, Wk, Wv, Wo, mask, mode)
    res = bass_utils.run_bass_kernel_spmd(
        nc, in_maps, core_ids=list(range(NCORES)), trace=trace)
    out = np.empty((B, S, D), np.float32)
    for b in range(B):
        acc = (res.results[2 * b]["outT"].astype(np.float32)
               + res.results[2 * b]["outB"].astype(np.float32)
               + res.results[2 * b + 1]["outT"].astype(np.float32)
               + res.results[2 * b + 1]["outB"].astype(np.float32))
        out[b] = acc.T
    return out, res


def kernel(x, Wq, Wk, Wv, Wo, mask):
    x = np.asarray(x, dtype=np.float32)
    Wq = np.asarray(Wq, dtype=np.float32)
    Wk = np.asarray(Wk, dtype=np.float32)
    Wv = np.asarray(Wv, dtype=np.float32)
    Wo = np.asarray(Wo, dtype=np.float32)
    out, _ = run(x, Wq, Wk, Wv, Wo, mask)
    return out
